# revision 3
# baseline (speedup 1.0000x reference)
"""Trainium2 Bass kernel for nn_AirspaceModel (126 per-node 2-layer LSTMs).

Sharding: 126 nodes padded to 128, 16 nodes per core across 8 cores
(expert-parallel over the independent per-node LSTMs). Each core keeps its
16 nodes' weights resident in SBUF and runs the full T=24 recurrence for
B=64.

Layout ("orientation A"): recurrent states live as [H, B] tiles (H on
partitions, batch on the free dim). Gates are computed per (node, gate) as
[100, 64] PSUM matmuls with lhsT = W^T slices; biases are folded in via a
constant ones-row appended to the h1 state tile. Elementwise LSTM math is
batched across 8-node groups with strided access patterns.

The linear+conv head is a weighted sum over (t, h) of layer-2 outputs:
out[n,b] = sum_t sum_h w_end[t] * w_lin[h] * h2[t,h,n,b] + const. h2 is
streamed to DRAM each step; a post-loop phase accumulates the head with
PSUM-accumulating matmuls against host-precomputed w_lin*w_end columns.
"""

import os
import sys

import numpy as np

for _p in ("/opt/trn_rl_repo", "/root/.axon_site/_ro/trn_rl_repo"):
    if os.path.isdir(_p) and _p not in sys.path:
        sys.path.append(_p)

import concourse.bass as bass
import concourse.mybir as mybir
import concourse.tile as tile
from concourse import bacc
from concourse.bass_utils import run_bass_kernel_spmd

F32 = mybir.dt.float32
AF = mybir.ActivationFunctionType

N_NODES, SEQ, FEAT, HID, B = 126, 24, 17, 100, 64
NCORES = 8
NP = 128            # padded node count
NN = NP // NCORES   # nodes per core (16)
GRP = 8             # nodes per elementwise group
NGRP = NN // GRP    # groups per core (2)
H1 = HID + 1        # h1 rows + ones row
NB = NN * B         # 1024

LAST_EXEC_TIME_NS = None
_PROG_CACHE = {}


def _build_program():
    nc = bacc.Bacc("TRN2", target_bir_lowering=False, debug=False)

    # --- DRAM I/O ---------------------------------------------------------
    xT = nc.dram_tensor("xT", [SEQ, FEAT, NB], F32, kind="ExternalInput")
    w0x = nc.dram_tensor("w0x", [FEAT, NN * 400], F32, kind="ExternalInput")
    w0h = nc.dram_tensor("w0h", [H1, NN * 400], F32, kind="ExternalInput")
    w1h1 = nc.dram_tensor("w1h1", [H1, NN * 400], F32, kind="ExternalInput")
    w1h2 = nc.dram_tensor("w1h2", [HID, NN * 400], F32, kind="ExternalInput")
    wlw = nc.dram_tensor("wlw", [HID, SEQ], F32, kind="ExternalInput")
    cst = nc.dram_tensor("cst", [1, 1], F32, kind="ExternalInput")

    outp = nc.dram_tensor("outp", [1, NB], F32, kind="ExternalOutput")
    h1n = nc.dram_tensor("h1n", [HID, NB], F32, kind="ExternalOutput")
    c1n = nc.dram_tensor("c1n", [HID, NB], F32, kind="ExternalOutput")
    c2n = nc.dram_tensor("c2n", [HID, NB], F32, kind="ExternalOutput")
    h2all = nc.dram_tensor("h2all", [SEQ, HID, NB], F32, kind="ExternalOutput")

    with tile.TileContext(nc) as tc:
        with (
            tc.tile_pool(name="wpool", bufs=1) as wpool,
            tc.tile_pool(name="state", bufs=1) as state,
            tc.tile_pool(name="xin", bufs=3) as xin,
            tc.tile_pool(name="ew", bufs=2) as ew,
        ):
            # --- resident weights ----------------------------------------
            w0x_sb = wpool.tile([FEAT, NN * 400], F32)
            w0h_sb = wpool.tile([H1, NN * 400], F32)
            w1h1_sb = wpool.tile([H1, NN * 400], F32)
            w1h2_sb = wpool.tile([HID, NN * 400], F32)
            wlw_sb = wpool.tile([HID, SEQ], F32)
            cst_sb = wpool.tile([1, 1], F32)
            nc.sync.dma_start(w0x_sb[:], w0x[:])
            nc.sync.dma_start(w0h_sb[:], w0h[:])
            nc.sync.dma_start(w1h1_sb[:], w1h1[:])
            nc.sync.dma_start(w1h2_sb[:], w1h2[:])
            nc.sync.dma_start(wlw_sb[:], wlw[:])
            nc.sync.dma_start(cst_sb[:], cst[:])

            # --- persistent state ----------------------------------------
            h1o = state.tile([H1, NB], F32)   # rows 0..99 h1, row 100 ones
            h2 = state.tile([HID, NB], F32)
            c1 = state.tile([HID, NB], F32)
            c2 = state.tile([HID, NB], F32)
            # compute APs must start at a 32-aligned partition: set the
            # ones-row by memsetting [0:101]=1 then [0:100]=0
            nc.vector.memset(h1o[0:H1, :], 1.0)
            nc.vector.memset(h1o[0:HID, :], 0.0)
            nc.vector.memset(h2[:], 0.0)
            nc.vector.memset(c1[:], 0.0)
            nc.vector.memset(c2[:], 0.0)

            def lstm_layer(g, gates, terms, c, hdst):
                """One layer for one 8-node group.

                gates: PSUM tile [100, GRP*256]; node j's gate q lands at
                cols j*256+q*64. terms: list of (w_sb, rhs, rhs_k) matmul
                contributions accumulated per gate; rhs is indexed at the
                global node's B-column slice. c: [100, GRP*B] slice of the
                cell-state home. hdst: [100, GRP*B] destination for h.
                """
                for j in range(GRP):
                    n = g * GRP + j
                    for q in range(4):
                        dst = gates[:, j * 256 + q * 64 : j * 256 + (q + 1) * 64]
                        wcol = n * 400 + q * 100
                        for ti, (w_sb, rhs) in enumerate(terms):
                            nc.tensor.matmul(
                                dst,
                                w_sb[:, wcol : wcol + 100],
                                rhs[:, n * B : (n + 1) * B],
                                start=(ti == 0),
                                stop=(ti == len(terms) - 1),
                            )
                # elementwise over the whole group
                g3 = gates.rearrange("p (n c) -> p n c", n=GRP)
                sif = ew.tile([HID, GRP * 128], F32, tag="sif")
                sif3 = sif.rearrange("p (n c) -> p n c", n=GRP)
                tg = ew.tile([HID, GRP * B], F32, tag="tg")
                so = ew.tile([HID, GRP * B], F32, tag="so")
                nc.scalar.activation(sif3[:], g3[:, :, 0:128], AF.Sigmoid)
                nc.scalar.activation(
                    tg.rearrange("p (n c) -> p n c", n=GRP)[:],
                    g3[:, :, 128:192],
                    AF.Tanh,
                )
                nc.scalar.activation(
                    so.rearrange("p (n c) -> p n c", n=GRP)[:],
                    g3[:, :, 192:256],
                    AF.Sigmoid,
                )
                tmp1 = ew.tile([HID, GRP * B], F32, tag="tmp1")
                tmp2 = ew.tile([HID, GRP * B], F32, tag="tmp2")
                tcn = ew.tile([HID, GRP * B], F32, tag="tcn")
                # c_new = sig(f)*c + sig(i)*tanh(g)
                nc.vector.tensor_mul(tmp1[:], sif3[:, :, 64:128], c)
                nc.vector.tensor_mul(tmp2[:], sif3[:, :, 0:64], tg[:])
                nc.vector.tensor_add(c, tmp1[:], tmp2[:])
                nc.scalar.activation(tcn[:], c, AF.Tanh)
                # h = sig(o)*tanh(c_new)
                nc.vector.tensor_mul(hdst, so[:], tcn[:])

            with tc.tile_pool(name="gates", bufs=1, space="PSUM") as gpsum:
                for t in range(SEQ):
                    xt_sb = xin.tile([FEAT, NB], F32, tag="xt")
                    nc.sync.dma_start(xt_sb[:], xT[t])
                    for g in range(NGRP):
                        cols = slice(g * GRP * B, (g + 1) * GRP * B)
                        gl0 = gpsum.tile([HID, GRP * 256], F32, tag="gl0")
                        lstm_layer(
                            g, gl0,
                            [(w0x_sb, xt_sb), (w0h_sb, h1o[0:H1, :])],
                            c1[:, cols],
                            h1o[0:HID, cols],
                        )
                        gl1 = gpsum.tile([HID, GRP * 256], F32, tag="gl1")
                        lstm_layer(
                            g, gl1,
                            [(w1h1_sb, h1o[0:H1, :]), (w1h2_sb, h2)],
                            c2[:, cols],
                            h2[:, cols],
                        )
                        nc.sync.dma_start(h2all[t][:, cols], h2[:, cols])

            # --- head: out[n,b] = sum_t wlw[:,t] . h2all[t] + cst ---------
            with tc.tile_pool(name="hps", bufs=1, space="PSUM") as hps:
                hp = hps.tile([1, NB], F32)
                for t in range(SEQ):
                    h2c = xin.tile([HID, NB], F32, tag="h2c")
                    nc.sync.dma_start(h2c[:], h2all[t])
                    for half in range(2):
                        nc.tensor.matmul(
                            hp[:, half * 512 : (half + 1) * 512],
                            wlw_sb[:, t : t + 1],
                            h2c[:, half * 512 : (half + 1) * 512],
                            start=(t == 0),
                            stop=(t == SEQ - 1),
                        )
                out_sb = ew.tile([1, NB], F32, tag="out_sb")
                nc.scalar.activation(
                    out_sb[:], hp[:], AF.Identity, bias=cst_sb[0:1, 0:1]
                )
                nc.sync.dma_start(outp[:], out_sb[:])

            # --- final states --------------------------------------------
            nc.sync.dma_start(h1n[:], h1o[0:HID, :])
            nc.sync.dma_start(c1n[:], c1[:])
            nc.sync.dma_start(c2n[:], c2[:])

    nc.compile()
    return nc


def _host_prep(x, W_ih0, W_hh0, b_ih0, b_hh0, W_ih1, W_hh1, b_ih1, b_hh1,
               w_lin, b_lin, w_end, b_end):
    """Pad to 128 nodes and build per-core input maps."""
    def pad_nodes(a):
        pad = [(0, 0)] * a.ndim
        pad[0] = (0, NP - N_NODES)
        return np.pad(a, pad)

    xp = np.pad(x, [(0, 0), (0, 0), (0, NP - N_NODES), (0, 0)])  # [B,T,NP,F]
    Wih0, Whh0 = pad_nodes(W_ih0), pad_nodes(W_hh0)
    Wih1, Whh1 = pad_nodes(W_ih1), pad_nodes(W_hh1)
    b0 = pad_nodes(b_ih0 + b_hh0)
    b1 = pad_nodes(b_ih1 + b_hh1)

    wlw = np.outer(w_lin[0], w_end[0]).astype(np.float32)  # [H, T]
    cst = np.array([[b_lin[0] * w_end[0].sum() + b_end[0]]], dtype=np.float32)

    in_maps = []
    for c in range(NCORES):
        sl = slice(c * NN, (c + 1) * NN)
        xT = xp[:, :, sl, :].transpose(1, 3, 2, 0).reshape(SEQ, FEAT, NB)
        w0x = Wih0[sl].transpose(2, 0, 1).reshape(FEAT, NN * 400)
        w0h = np.concatenate(
            [Whh0[sl].transpose(2, 0, 1), b0[sl][None]], axis=0
        ).reshape(H1, NN * 400)
        w1h1 = np.concatenate(
            [Wih1[sl].transpose(2, 0, 1), b1[sl][None]], axis=0
        ).reshape(H1, NN * 400)
        w1h2 = Whh1[sl].transpose(2, 0, 1).reshape(HID, NN * 400)
        in_maps.append({
            "xT": np.ascontiguousarray(xT, dtype=np.float32),
            "w0x": np.ascontiguousarray(w0x, dtype=np.float32),
            "w0h": np.ascontiguousarray(w0h, dtype=np.float32),
            "w1h1": np.ascontiguousarray(w1h1, dtype=np.float32),
            "w1h2": np.ascontiguousarray(w1h2, dtype=np.float32),
            "wlw": wlw,
            "cst": cst,
        })
    return in_maps


def kernel(x, W_ih0, W_hh0, b_ih0, b_hh0, W_ih1, W_hh1, b_ih1, b_hh1,
           w_lin, b_lin, w_end, b_end):
    global LAST_EXEC_TIME_NS
    args = (x, W_ih0, W_hh0, b_ih0, b_hh0, W_ih1, W_hh1, b_ih1, b_hh1,
            w_lin, b_lin, w_end, b_end)
    args = tuple(np.asarray(a, dtype=np.float32) for a in args)
    in_maps = _host_prep(*args)

    if "prog" not in _PROG_CACHE:
        _PROG_CACHE["prog"] = _build_program()
    nc = _PROG_CACHE["prog"]

    trace = os.environ.get("KERNEL_TRACE", "0") == "1"
    res = run_bass_kernel_spmd(
        nc, in_maps, core_ids=list(range(NCORES)), trace=trace
    )
    LAST_EXEC_TIME_NS = res.exec_time_ns

    out = np.zeros((B, 1, N_NODES, 1), dtype=np.float32)
    hn = np.zeros((N_NODES, 2, B, HID), dtype=np.float32)
    cn = np.zeros((N_NODES, 2, B, HID), dtype=np.float32)
    for c in range(NCORES):
        r = res.results[c]
        o = r["outp"].reshape(NN, B)
        h1f = r["h1n"].reshape(HID, NN, B)
        c1f = r["c1n"].reshape(HID, NN, B)
        c2f = r["c2n"].reshape(HID, NN, B)
        h2f = r["h2all"].reshape(SEQ, HID, NN, B)[SEQ - 1]
        for j in range(NN):
            n = c * NN + j
            if n >= N_NODES:
                break
            out[:, 0, n, 0] = o[j]
            hn[n, 0] = h1f[:, j, :].T
            hn[n, 1] = h2f[:, j, :].T
            cn[n, 0] = c1f[:, j, :].T
            cn[n, 1] = c2f[:, j, :].T
    return (out, hn, cn)


# revision 5
# speedup vs baseline: 1.9774x; 1.9774x over previous
"""Trainium2 Bass kernel for nn_AirspaceModel (126 per-node 2-layer LSTMs).

Sharding: 126 nodes padded to 128, 16 nodes per core across 8 cores
(expert-parallel over the independent per-node LSTMs). Each core keeps its
16 nodes' weights resident in SBUF and runs the full T=24 recurrence for
B=64.

Matmul orientation: gates[b, 4H] = xcatT.T @ W^T — the (transposed)
activations are the stationary operand [K<=128, 64] and the per-node
weights stream as rhs [K, 400], so each matmul streams 400 columns
instead of 64 and weights are never loaded as stationary. Nodes are
processed in pairs: the even node's matmul writes PSUM partitions 0-63
(array column-groups 0-1) and the odd node's partitions 64-127 (groups
2-3, tile_position=(0,64)), so the two matmuls run concurrently and
elementwise ops see full 128-partition tiles.

Recurrent states live as [128, pair*101] tiles (batch-pair on partitions).
Each step the new h1/h2 are transposed back to [H, 128] via PE transpose
(h1 carries a built-in ones column so the transpose also produces the
bias row used to fold biases into the matmuls). The transposed h1 of step
t is reused as layer-0 stationary input of step t+1.

The linear+conv head is a weighted sum over (t, h) of layer-2 outputs;
transposed h2 is streamed to DRAM each step and a post-loop phase
accumulates out[n,b] with PSUM-accumulating matmuls against
host-precomputed w_lin[h]*w_end[t] columns.
"""

import os
import sys

import numpy as np

for _p in ("/opt/trn_rl_repo", "/root/.axon_site/_ro/trn_rl_repo"):
    if os.path.isdir(_p) and _p not in sys.path:
        sys.path.append(_p)

import concourse.bass as bass
import concourse.mybir as mybir
import concourse.tile as tile
from concourse import bacc
from concourse.bass_utils import run_bass_kernel_spmd
from concourse.masks import make_identity

F32 = mybir.dt.float32
AF = mybir.ActivationFunctionType

N_NODES, SEQ, FEAT, HID, B = 126, 24, 17, 100, 64
NCORES = 8
NP = 128             # padded node count
NN = NP // NCORES    # nodes per core (16)
PAIRS = NN // 2      # node pairs per core (8)
GP = 4               # pairs per pipeline group
NGRP = PAIRS // GP   # groups (2)
H1 = HID + 1         # h rows + ones col/row (101)
K0 = H1 + FEAT       # layer-0 contraction: h1 + ones + x = 118

LAST_EXEC_TIME_NS = None
_PROG_CACHE = {}


def _build_program():
    nc = bacc.Bacc("TRN2", target_bir_lowering=False, debug=False)

    xTp = nc.dram_tensor("xTp", [SEQ, PAIRS, FEAT, 128], F32, kind="ExternalInput")
    w0c = nc.dram_tensor("w0c", [K0, NN * 400], F32, kind="ExternalInput")
    w1c = nc.dram_tensor("w1c", [H1, NN * 400], F32, kind="ExternalInput")
    w1h2 = nc.dram_tensor("w1h2", [HID, NN * 400], F32, kind="ExternalInput")
    wlw = nc.dram_tensor("wlw", [HID, SEQ], F32, kind="ExternalInput")
    cst = nc.dram_tensor("cst", [1, 1], F32, kind="ExternalInput")
    ones_d = nc.dram_tensor("ones_d", [1, PAIRS * 128], F32, kind="ExternalInput")

    outp = nc.dram_tensor("outp", [1, PAIRS * 128], F32, kind="ExternalOutput")
    h1f = nc.dram_tensor("h1f", [128, PAIRS * HID], F32, kind="ExternalOutput")
    h2f = nc.dram_tensor("h2f", [128, PAIRS * HID], F32, kind="ExternalOutput")
    c1f = nc.dram_tensor("c1f", [128, PAIRS * HID], F32, kind="ExternalOutput")
    c2f = nc.dram_tensor("c2f", [128, PAIRS * HID], F32, kind="ExternalOutput")
    h2allT = nc.dram_tensor(
        "h2allT", [SEQ, HID, PAIRS * 128], F32, kind="ExternalOutput")

    with tile.TileContext(nc) as tc:
        with (
            tc.tile_pool(name="wpool", bufs=1) as wpool,
            tc.tile_pool(name="state", bufs=1) as state,
            tc.tile_pool(name="xc", bufs=1) as xc,
            tc.tile_pool(name="ew", bufs=3) as ew,
        ):
            # --- resident weights / constants ----------------------------
            w0c_sb = wpool.tile([K0, NN * 400], F32)
            w1c_sb = wpool.tile([H1, NN * 400], F32)
            w1h2_sb = wpool.tile([HID, NN * 400], F32)
            wlw_sb = wpool.tile([HID, SEQ], F32)
            cst_sb = wpool.tile([1, 1], F32)
            ident = wpool.tile([128, 128], F32)
            nc.sync.dma_start(w0c_sb[:], w0c[:])
            nc.sync.dma_start(w1c_sb[:], w1c[:])
            nc.sync.dma_start(w1h2_sb[:], w1h2[:])
            nc.sync.dma_start(wlw_sb[:], wlw[:])
            nc.sync.dma_start(cst_sb[:], cst[:])
            make_identity(nc, ident[:])

            # --- persistent state (batch-pair on partitions) -------------
            # h1p: [128, 8*101], pair p cols p*101..p*101+100 (col 100=ones)
            h1p = state.tile([128, PAIRS * H1], F32)
            h2p = state.tile([128, PAIRS * HID], F32)
            c1p = state.tile([128, PAIRS * HID], F32)
            c2p = state.tile([128, PAIRS * HID], F32)
            nc.vector.memset(h1p[:], 0.0)
            nc.vector.memset(
                h1p.rearrange("p (n c) -> p n c", c=H1)[:, :, HID : H1], 1.0)
            nc.vector.memset(h2p[:], 0.0)
            nc.vector.memset(c1p[:], 0.0)
            nc.vector.memset(c2p[:], 0.0)

            # transposed stationary inputs, one tile per pair, carried
            # across steps: xh1[p] = [118,128] rows 0-99 h1T, 100 ones,
            # 101-117 x_t; xh2[p] = [100,128] h2T
            xh1 = [None] * PAIRS
            xh2 = [None] * PAIRS
            for p in range(PAIRS):
                t0 = xc.tile([K0, 128], F32, tag="xh1", bufs=24, name=f"xh1_{p}")
                nc.vector.memset(t0[:], 0.0)
                nc.sync.dma_start(
                    t0[HID : H1, :], ones_d[:, p * 128 : (p + 1) * 128])
                nc.sync.dma_start(t0[H1:K0, :], xTp[0, p])
                xh1[p] = t0
                t1 = xc.tile([HID, 128], F32, tag="xh2", bufs=24, name=f"xh2_{p}")
                nc.vector.memset(t1[:], 0.0)
                xh2[p] = t1

            def ewise(gl, hview, cview, sfx):
                """LSTM elementwise for a 4-pair group.

                gl: PSUM gates [128, 4, 512] (cols 0-399 used per pair).
                hview: [128, 4, 100] h destination, cview: [128, 4, 100]
                cell-state home (read+write).
                """
                sf = ew.tile([128, GP * 200], F32, tag="sf" + sfx)
                sf3 = sf.rearrange("p (n c) -> p n c", n=GP)
                tg = ew.tile([128, GP * 100], F32, tag="tg" + sfx)
                tg3 = tg.rearrange("p (n c) -> p n c", n=GP)
                so = ew.tile([128, GP * 100], F32, tag="so" + sfx)
                so3 = so.rearrange("p (n c) -> p n c", n=GP)
                nc.scalar.activation(sf3[:], gl[:, :, 0:200], AF.Sigmoid)
                nc.scalar.activation(tg3[:], gl[:, :, 200:300], AF.Tanh)
                nc.scalar.activation(so3[:], gl[:, :, 300:400], AF.Sigmoid)
                tmp1 = ew.tile([128, GP * 100], F32, tag="t1" + sfx)
                t13 = tmp1.rearrange("p (n c) -> p n c", n=GP)
                tmp2 = ew.tile([128, GP * 100], F32, tag="t2" + sfx)
                t23 = tmp2.rearrange("p (n c) -> p n c", n=GP)
                tcn = ew.tile([128, GP * 100], F32, tag="tc" + sfx)
                tc3 = tcn.rearrange("p (n c) -> p n c", n=GP)
                # c_new = sig(f)*c + sig(i)*tanh(g)
                nc.gpsimd.tensor_mul(t13[:], sf3[:, :, 100:200], cview)
                nc.gpsimd.tensor_mul(t23[:], sf3[:, :, 0:100], tg3[:])
                nc.vector.tensor_add(cview, t13[:], t23[:])
                nc.scalar.activation(tc3[:], cview, AF.Tanh)
                # h = sig(o)*tanh(c_new)
                nc.vector.tensor_mul(hview, so3[:], tc3[:])

            h1p3 = h1p.rearrange("p (n c) -> p n c", c=H1)
            h2p3 = h2p.rearrange("p (n c) -> p n c", c=HID)
            c1p3 = c1p.rearrange("p (n c) -> p n c", c=HID)
            c2p3 = c2p.rearrange("p (n c) -> p n c", c=HID)

            with tc.tile_pool(name="gates", bufs=2, space="PSUM") as gpsum:
                for t in range(SEQ):
                    for g in range(NGRP):
                        prs = range(g * GP, (g + 1) * GP)
                        # --- layer 0 matmuls -------------------------
                        gl0 = gpsum.tile([128, GP, 512], F32, tag="g")
                        for j, p in enumerate(prs):
                            for s in range(2):
                                n = 2 * p + s
                                nc.tensor.matmul(
                                    gl0[s * 64 : (s + 1) * 64, j, 0:400],
                                    xh1[p][0:K0, s * 64 : (s + 1) * 64],
                                    w0c_sb[:, n * 400 : (n + 1) * 400],
                                    start=True, stop=True,
                                    tile_position=(0, s * 64),
                                )
                        # --- layer 0 elementwise ---------------------
                        ewise(
                            gl0,
                            h1p3[:, g * GP : (g + 1) * GP, 0:HID],
                            c1p3[:, g * GP : (g + 1) * GP, :],
                            "a",
                        )
                        # --- transpose new h1 (+ones) for L1 & next L0
                        tpa = gpsum.tile([128, GP, 512], F32, tag="g")
                        new_xh1 = []
                        for j, p in enumerate(prs):
                            nc.tensor.transpose(
                                tpa[0:H1, j, 0:128],
                                h1p[:, p * H1 : (p + 1) * H1],
                                ident[:],
                            )
                        for j, p in enumerate(prs):
                            nt = xc.tile([K0, 128], F32, tag="xh1", bufs=24,
                                         name=f"xh1_{t}_{p}")
                            nc.vector.tensor_copy(
                                nt[0:H1, :], tpa[0:H1, j, 0:128])
                            if t + 1 < SEQ:
                                nc.sync.dma_start(nt[H1:K0, :], xTp[t + 1, p])
                            new_xh1.append(nt)
                        # --- layer 1 matmuls -------------------------
                        gl1 = gpsum.tile([128, GP, 512], F32, tag="g")
                        for j, p in enumerate(prs):
                            for s in range(2):
                                n = 2 * p + s
                                sl = slice(s * 64, (s + 1) * 64)
                                nc.tensor.matmul(
                                    gl1[sl, j, 0:400],
                                    new_xh1[j][0:H1, sl],
                                    w1c_sb[:, n * 400 : (n + 1) * 400],
                                    start=True, stop=False,
                                    tile_position=(0, s * 64),
                                )
                                nc.tensor.matmul(
                                    gl1[sl, j, 0:400],
                                    xh2[p][0:HID, sl],
                                    w1h2_sb[:, n * 400 : (n + 1) * 400],
                                    start=False, stop=True,
                                    tile_position=(0, s * 64),
                                )
                        # --- layer 1 elementwise ---------------------
                        ewise(
                            gl1,
                            h2p3[:, g * GP : (g + 1) * GP, :],
                            c2p3[:, g * GP : (g + 1) * GP, :],
                            "b",
                        )
                        # --- transpose new h2; stream h2T to DRAM ----
                        tpb = gpsum.tile([128, GP, 512], F32, tag="g")
                        for j, p in enumerate(prs):
                            nc.tensor.transpose(
                                tpb[0:HID, j, 0:128],
                                h2p[:, p * HID : (p + 1) * HID],
                                ident[:],
                            )
                        for j, p in enumerate(prs):
                            nt = xc.tile([HID, 128], F32, tag="xh2", bufs=24,
                                         name=f"xh2_{t}_{p}")
                            nc.vector.tensor_copy(nt[:], tpb[0:HID, j, 0:128])
                            nc.sync.dma_start(
                                h2allT[t][:, p * 128 : (p + 1) * 128], nt[:])
                            xh2[p] = nt
                        for j, p in enumerate(prs):
                            xh1[p] = new_xh1[j]

            # --- head: out = sum_t wlw[:,t] . h2allT[t] + cst ------------
            with tc.tile_pool(name="hps", bufs=1, space="PSUM") as hps:
                hp = hps.tile([1, PAIRS * 128], F32)
                for t in range(SEQ):
                    h2c = ew.tile([HID, PAIRS * 128], F32, tag="h2c")
                    nc.sync.dma_start(h2c[:], h2allT[t])
                    for half in range(2):
                        nc.tensor.matmul(
                            hp[:, half * 512 : (half + 1) * 512],
                            wlw_sb[:, t : t + 1],
                            h2c[:, half * 512 : (half + 1) * 512],
                            start=(t == 0),
                            stop=(t == SEQ - 1),
                        )
                out_sb = ew.tile([1, PAIRS * 128], F32, tag="out_sb")
                nc.scalar.activation(
                    out_sb[:], hp[:], AF.Identity, bias=cst_sb[0:1, 0:1])
                nc.sync.dma_start(outp[:], out_sb[:])

            # --- final states --------------------------------------------
            nc.sync.dma_start(h1f[:], h1p3[:, :, 0:HID])
            nc.sync.dma_start(h2f[:], h2p[:])
            nc.sync.dma_start(c1f[:], c1p[:])
            nc.sync.dma_start(c2f[:], c2p[:])

    nc.compile()
    return nc


def _host_prep(x, W_ih0, W_hh0, b_ih0, b_hh0, W_ih1, W_hh1, b_ih1, b_hh1,
               w_lin, b_lin, w_end, b_end):
    """Pad to 128 nodes and build per-core input maps."""
    def pad_nodes(a):
        pad = [(0, 0)] * a.ndim
        pad[0] = (0, NP - N_NODES)
        return np.pad(a, pad)

    xp = np.pad(x, [(0, 0), (0, 0), (0, NP - N_NODES), (0, 0)])  # [B,T,NP,F]
    Wih0, Whh0 = pad_nodes(W_ih0), pad_nodes(W_hh0)
    Wih1, Whh1 = pad_nodes(W_ih1), pad_nodes(W_hh1)
    b0 = pad_nodes(b_ih0 + b_hh0)
    b1 = pad_nodes(b_ih1 + b_hh1)

    wlw = np.outer(w_lin[0], w_end[0]).astype(np.float32)  # [H, T]
    cst = np.array([[b_lin[0] * w_end[0].sum() + b_end[0]]], dtype=np.float32)
    ones = np.ones((1, PAIRS * 128), dtype=np.float32)

    in_maps = []
    for c in range(NCORES):
        sl = slice(c * NN, (c + 1) * NN)
        # xTp[t, p, f, s*64+b] = x[b, t, node, f], node = 16c + 2p + s
        xTp = (xp[:, :, sl, :]                 # [B, T, 16, F]
               .transpose(1, 2, 3, 0)          # [T, 16, F, B]
               .reshape(SEQ, PAIRS, 2, FEAT, B)
               .transpose(0, 1, 3, 2, 4)       # [T, P, F, 2, B]
               .reshape(SEQ, PAIRS, FEAT, 128))
        w0 = np.concatenate(
            [Whh0[sl].transpose(2, 0, 1), b0[sl][None],
             Wih0[sl].transpose(2, 0, 1)], axis=0).reshape(K0, NN * 400)
        w1 = np.concatenate(
            [Wih1[sl].transpose(2, 0, 1), b1[sl][None]], axis=0
        ).reshape(H1, NN * 400)
        wh2 = Whh1[sl].transpose(2, 0, 1).reshape(HID, NN * 400)
        in_maps.append({
            "xTp": np.ascontiguousarray(xTp, dtype=np.float32),
            "w0c": np.ascontiguousarray(w0, dtype=np.float32),
            "w1c": np.ascontiguousarray(w1, dtype=np.float32),
            "w1h2": np.ascontiguousarray(wh2, dtype=np.float32),
            "wlw": wlw,
            "cst": cst,
            "ones_d": ones,
        })
    return in_maps


def kernel(x, W_ih0, W_hh0, b_ih0, b_hh0, W_ih1, W_hh1, b_ih1, b_hh1,
           w_lin, b_lin, w_end, b_end):
    global LAST_EXEC_TIME_NS
    args = (x, W_ih0, W_hh0, b_ih0, b_hh0, W_ih1, W_hh1, b_ih1, b_hh1,
            w_lin, b_lin, w_end, b_end)
    args = tuple(np.asarray(a, dtype=np.float32) for a in args)
    in_maps = _host_prep(*args)

    if "prog" not in _PROG_CACHE:
        _PROG_CACHE["prog"] = _build_program()
    nc = _PROG_CACHE["prog"]

    trace = os.environ.get("KERNEL_TRACE", "0") == "1"
    res = run_bass_kernel_spmd(
        nc, in_maps, core_ids=list(range(NCORES)), trace=trace
    )
    LAST_EXEC_TIME_NS = res.exec_time_ns

    out = np.zeros((B, 1, N_NODES, 1), dtype=np.float32)
    hn = np.zeros((N_NODES, 2, B, HID), dtype=np.float32)
    cn = np.zeros((N_NODES, 2, B, HID), dtype=np.float32)
    for c in range(NCORES):
        r = res.results[c]
        nlo, nhi = c * NN, min((c + 1) * NN, N_NODES)
        cnt = nhi - nlo

        # outp: [1, p*128 + s*64 + b] -> [node, b]
        o = r["outp"].reshape(PAIRS, 2, B).reshape(NN, B)
        out[:, 0, nlo:nhi, 0] = o[:cnt].T

        def states(a):  # [128, P*100] -> [node, b, h]
            v = a.reshape(2, B, PAIRS, HID)       # [s, b, p, h]
            return v.transpose(2, 0, 1, 3).reshape(NN, B, HID)

        hn[nlo:nhi, 0] = states(r["h1f"])[:cnt]
        hn[nlo:nhi, 1] = states(r["h2f"])[:cnt]
        cn[nlo:nhi, 0] = states(r["c1f"])[:cnt]
        cn[nlo:nhi, 1] = states(r["c2f"])[:cnt]
    return (out, hn, cn)


# revision 7
# speedup vs baseline: 3.1668x; 1.6015x over previous
"""Trainium2 Bass kernel for nn_AirspaceModel (126 per-node 2-layer LSTMs).

Sharding: 126 nodes padded to 128, 16 nodes per core across 8 cores
(expert-parallel over the independent per-node LSTMs). Each core keeps its
16 nodes' weights resident in SBUF and runs the full T=24 recurrence for
B=64.

Matmul orientation: gates[b, 4H] = xcatT.T @ W^T — the (transposed)
activations are the stationary operand [K<=128, 64] and the per-node
weights stream as rhs [K, 400] in float32r (full-rate 4-byte streaming,
vs 4 cycles/row for plain fp32). Nodes are processed in pairs: the even
node's matmul writes PSUM partitions 0-63 and the odd node's partitions
64-127 (tile_position=(0,64)), so the two matmuls run concurrently in
separate array column groups and elementwise ops see full 128-partition
tiles.

Recurrent states live as [128, pair*101] tiles (batch-pair on partitions).
Each step the new h1/h2 are transposed back to [H, 128] via PE transpose
(h1 carries a built-in ones column so the transpose also produces the
bias row used to fold biases into the matmuls). The transposed h1 of step
t is reused as layer-0 stationary input of step t+1.

The linear+conv head is a weighted sum over (t, h) of layer-2 outputs;
transposed h2 is streamed to DRAM each step and a post-loop phase
accumulates out[n,b] with PSUM-accumulating matmuls against
host-precomputed w_lin[h]*w_end[t] columns.
"""

import os
import sys

import numpy as np
import ml_dtypes

for _p in ("/opt/trn_rl_repo", "/root/.axon_site/_ro/trn_rl_repo"):
    if os.path.isdir(_p) and _p not in sys.path:
        sys.path.append(_p)

import concourse.bass as bass
import concourse.mybir as mybir
import concourse.tile as tile
from concourse import bacc
from concourse.bass_utils import run_bass_kernel_spmd
from concourse.masks import make_identity

F32 = mybir.dt.float32
BF16 = mybir.dt.bfloat16
AF = mybir.ActivationFunctionType

N_NODES, SEQ, FEAT, HID, B = 126, 24, 17, 100, 64
NCORES = 8
NP = 128             # padded node count
NN = NP // NCORES    # nodes per core (16)
PAIRS = NN // 2      # node pairs per core (8)
GP = 4               # pairs per pipeline group
NGRP = PAIRS // GP   # groups (2)
H1 = HID + 1         # h rows + ones col/row (101)
K0 = H1 + FEAT       # layer-0 contraction: h1 + ones + x = 118

LAST_EXEC_TIME_NS = None
_PROG_CACHE = {}


def _build_program():
    nc = bacc.Bacc("TRN2", target_bir_lowering=False, debug=False)

    xTp = nc.dram_tensor("xTp", [SEQ, FEAT, PAIRS * 128], BF16, kind="ExternalInput")
    w0c = nc.dram_tensor("w0c", [K0, NN * 400], BF16, kind="ExternalInput")
    w1c = nc.dram_tensor("w1c", [H1, NN * 400], BF16, kind="ExternalInput")
    w1h2 = nc.dram_tensor("w1h2", [HID, NN * 400], BF16, kind="ExternalInput")
    wlw = nc.dram_tensor("wlw", [HID, SEQ], BF16, kind="ExternalInput")
    cst = nc.dram_tensor("cst", [1, 1], F32, kind="ExternalInput")
    ones_d = nc.dram_tensor("ones_d", [1, PAIRS * 128], BF16, kind="ExternalInput")

    outp = nc.dram_tensor("outp", [1, PAIRS * 128], F32, kind="ExternalOutput")
    h1f = nc.dram_tensor("h1f", [128, PAIRS * HID], F32, kind="ExternalOutput")
    h2f = nc.dram_tensor("h2f", [128, PAIRS * HID], F32, kind="ExternalOutput")
    c1f = nc.dram_tensor("c1f", [128, PAIRS * HID], F32, kind="ExternalOutput")
    c2f = nc.dram_tensor("c2f", [128, PAIRS * HID], F32, kind="ExternalOutput")
    h2allT = nc.dram_tensor(
        "h2allT", [SEQ, HID, PAIRS * 128], BF16, kind="ExternalOutput")

    with tile.TileContext(nc) as tc:
        with (
            tc.tile_pool(name="wpool", bufs=1) as wpool,
            tc.tile_pool(name="state", bufs=1) as state,
            tc.tile_pool(name="xc", bufs=3) as xc,
            tc.tile_pool(name="ew", bufs=3) as ew,
        ):
            # --- resident weights / constants ----------------------------
            w0c_sb = wpool.tile([K0, NN * 400], BF16)
            w1c_sb = wpool.tile([H1, NN * 400], BF16)
            w1h2_sb = wpool.tile([HID, NN * 400], BF16)
            wlw_sb = wpool.tile([HID, SEQ], BF16)
            cst_sb = wpool.tile([1, 1], F32)
            ident = wpool.tile([128, 128], F32)
            nc.sync.dma_start(w0c_sb[:], w0c[:])
            nc.sync.dma_start(w1c_sb[:], w1c[:])
            nc.sync.dma_start(w1h2_sb[:], w1h2[:])
            nc.sync.dma_start(wlw_sb[:], wlw[:])
            nc.sync.dma_start(cst_sb[:], cst[:])
            make_identity(nc, ident[:])

            # --- persistent state (batch-pair on partitions) -------------
            # h1p: [128, 8*101], pair p cols p*101..p*101+100 (col 100=ones)
            h1p = state.tile([128, PAIRS * H1], F32)
            h2p = state.tile([128, PAIRS * HID], F32)
            c1p = state.tile([128, PAIRS * HID], F32)
            c2p = state.tile([128, PAIRS * HID], F32)
            nc.vector.memset(h1p[:], 0.0)
            nc.vector.memset(
                h1p.rearrange("p (n c) -> p n c", c=H1)[:, :, HID : H1], 1.0)
            nc.vector.memset(h2p[:], 0.0)
            nc.vector.memset(c1p[:], 0.0)
            nc.vector.memset(c2p[:], 0.0)

            # transposed stationary inputs, one [K, 8, 128] tile per step:
            # xh1 rows 0-99 h1T, 100 ones, 101-117 x_t; xh2 = h2T
            xh1_prev = xc.tile([K0, PAIRS, 128], BF16, tag="xh1", name="xh1_init")
            nc.vector.memset(xh1_prev[:], 0.0)
            nc.sync.dma_start(
                xh1_prev[HID : H1].rearrange("o p b -> o (p b)"), ones_d[:])
            nc.sync.dma_start(
                xh1_prev[H1:K0].rearrange("o p b -> o (p b)"), xTp[0])
            xh2_prev = xc.tile([HID, PAIRS, 128], BF16, tag="xh2", name="xh2_init")
            nc.vector.memset(xh2_prev[:], 0.0)

            def ewise(gl, hview, cview, sfx):
                """LSTM elementwise for a 4-pair group.

                gl: PSUM gates [128, 4, 512] (cols 0-399 used per pair).
                hview/cview: [128, 4, 100] h destination / cell-state home.
                """
                sf = ew.tile([128, GP * 200], F32, tag="sf" + sfx)
                sf3 = sf.rearrange("p (n c) -> p n c", n=GP)
                tg = ew.tile([128, GP * 100], F32, tag="tg" + sfx)
                tg3 = tg.rearrange("p (n c) -> p n c", n=GP)
                so = ew.tile([128, GP * 100], F32, tag="so" + sfx)
                so3 = so.rearrange("p (n c) -> p n c", n=GP)
                nc.scalar.activation(sf3[:], gl[:, :, 0:200], AF.Sigmoid)
                nc.scalar.activation(tg3[:], gl[:, :, 200:300], AF.Tanh)
                nc.scalar.activation(so3[:], gl[:, :, 300:400], AF.Sigmoid)
                tmp1 = ew.tile([128, GP * 100], F32, tag="t1" + sfx)
                t13 = tmp1.rearrange("p (n c) -> p n c", n=GP)
                tmp2 = ew.tile([128, GP * 100], F32, tag="t2" + sfx)
                t23 = tmp2.rearrange("p (n c) -> p n c", n=GP)
                tcn = ew.tile([128, GP * 100], F32, tag="tc" + sfx)
                tc3 = tcn.rearrange("p (n c) -> p n c", n=GP)
                # c_new = sig(f)*c + sig(i)*tanh(g)
                nc.gpsimd.tensor_mul(t13[:], sf3[:, :, 100:200], cview)
                nc.gpsimd.tensor_mul(t23[:], sf3[:, :, 0:100], tg3[:])
                nc.vector.tensor_add(cview, t13[:], t23[:])
                nc.scalar.activation(tc3[:], cview, AF.Tanh)
                # h = sig(o)*tanh(c_new)
                nc.vector.tensor_mul(hview, so3[:], tc3[:])

            h1p3 = h1p.rearrange("p (n c) -> p n c", c=H1)
            h2p3 = h2p.rearrange("p (n c) -> p n c", c=HID)
            c1p3 = c1p.rearrange("p (n c) -> p n c", c=HID)
            c2p3 = c2p.rearrange("p (n c) -> p n c", c=HID)

            with tc.tile_pool(name="gates", bufs=2, space="PSUM") as gpsum:
                for t in range(SEQ):
                    xh1_new = xc.tile([K0, PAIRS, 128], BF16, tag="xh1",
                                      name=f"xh1_{t}")
                    if t + 1 < SEQ:
                        nc.sync.dma_start(
                            xh1_new[H1:K0].rearrange("o p b -> o (p b)"),
                            xTp[t + 1])
                    xh2_new = xc.tile([HID, PAIRS, 128], BF16, tag="xh2",
                                      name=f"xh2_{t}")
                    for g in range(NGRP):
                        prs = range(g * GP, (g + 1) * GP)
                        # --- layer 0 matmuls -------------------------
                        gl0 = gpsum.tile([128, GP, 512], F32, tag="g")
                        for j, p in enumerate(prs):
                            for s in range(2):
                                n = 2 * p + s
                                nc.tensor.matmul(
                                    gl0[s * 64 : (s + 1) * 64, j, 0:400],
                                    xh1_prev[0:K0, p, s * 64 : (s + 1) * 64],
                                    w0c_sb[:, n * 400 : (n + 1) * 400],
                                    start=True, stop=True,
                                    tile_position=(0, s * 64),
                                )
                        # --- layer 0 elementwise ---------------------
                        ewise(
                            gl0,
                            h1p3[:, g * GP : (g + 1) * GP, 0:HID],
                            c1p3[:, g * GP : (g + 1) * GP, :],
                            "a",
                        )
                        # --- transpose new h1 (+ones) for L1 & next L0
                        tpa = gpsum.tile([128, GP, 512], F32, tag="g")
                        for j, p in enumerate(prs):
                            nc.tensor.transpose(
                                tpa[0:H1, j, 0:128],
                                h1p[:, p * H1 : (p + 1) * H1],
                                ident[:],
                            )
                        for j, p in enumerate(prs):
                            nc.vector.tensor_copy(
                                xh1_new[0:H1, p, :], tpa[0:H1, j, 0:128])
                        # --- layer 1 matmuls -------------------------
                        gl1 = gpsum.tile([128, GP, 512], F32, tag="g")
                        for j, p in enumerate(prs):
                            for s in range(2):
                                n = 2 * p + s
                                sl = slice(s * 64, (s + 1) * 64)
                                nc.tensor.matmul(
                                    gl1[sl, j, 0:400],
                                    xh1_new[0:H1, p, sl],
                                    w1c_sb[:, n * 400 : (n + 1) * 400],
                                    start=True, stop=False,
                                    tile_position=(0, s * 64),
                                )
                                nc.tensor.matmul(
                                    gl1[sl, j, 0:400],
                                    xh2_prev[0:HID, p, sl],
                                    w1h2_sb[:, n * 400 : (n + 1) * 400],
                                    start=False, stop=True,
                                    tile_position=(0, s * 64),
                                )
                        # --- layer 1 elementwise ---------------------
                        ewise(
                            gl1,
                            h2p3[:, g * GP : (g + 1) * GP, :],
                            c2p3[:, g * GP : (g + 1) * GP, :],
                            "b",
                        )
                        # --- transpose new h2 ------------------------
                        tpb = gpsum.tile([128, GP, 512], F32, tag="g")
                        for j, p in enumerate(prs):
                            nc.tensor.transpose(
                                tpb[0:HID, j, 0:128],
                                h2p[:, p * HID : (p + 1) * HID],
                                ident[:],
                            )
                        for j, p in enumerate(prs):
                            nc.vector.tensor_copy(
                                xh2_new[0:HID, p, :], tpb[0:HID, j, 0:128])
                    # stream h2T of this step to DRAM for the head
                    nc.sync.dma_start(
                        h2allT[t], xh2_new.rearrange("o p b -> o (p b)"))
                    xh1_prev = xh1_new
                    xh2_prev = xh2_new

            # --- head: out = sum_t wlw[:,t] . h2allT[t] + cst ------------
            with tc.tile_pool(name="hps", bufs=1, space="PSUM") as hps:
                hp = hps.tile([1, PAIRS * 128], F32)
                for t in range(SEQ):
                    h2c = ew.tile([HID, PAIRS * 128], BF16, tag="h2c")
                    nc.sync.dma_start(h2c[:], h2allT[t])
                    for half in range(2):
                        nc.tensor.matmul(
                            hp[:, half * 512 : (half + 1) * 512],
                            wlw_sb[:, t : t + 1],
                            h2c[:, half * 512 : (half + 1) * 512],
                            start=(t == 0),
                            stop=(t == SEQ - 1),
                        )
                out_sb = ew.tile([1, PAIRS * 128], F32, tag="out_sb")
                nc.scalar.activation(
                    out_sb[:], hp[:], AF.Identity, bias=cst_sb[0:1, 0:1])
                nc.sync.dma_start(outp[:], out_sb[:])

            # --- final states --------------------------------------------
            nc.sync.dma_start(h1f[:], h1p3[:, :, 0:HID])
            nc.sync.dma_start(h2f[:], h2p[:])
            nc.sync.dma_start(c1f[:], c1p[:])
            nc.sync.dma_start(c2f[:], c2p[:])

    nc.compile()
    return nc


def _host_prep(x, W_ih0, W_hh0, b_ih0, b_hh0, W_ih1, W_hh1, b_ih1, b_hh1,
               w_lin, b_lin, w_end, b_end):
    """Pad to 128 nodes and build per-core input maps."""
    def pad_nodes(a):
        pad = [(0, 0)] * a.ndim
        pad[0] = (0, NP - N_NODES)
        return np.pad(a, pad)

    xp = np.pad(x, [(0, 0), (0, 0), (0, NP - N_NODES), (0, 0)])  # [B,T,NP,F]
    Wih0, Whh0 = pad_nodes(W_ih0), pad_nodes(W_hh0)
    Wih1, Whh1 = pad_nodes(W_ih1), pad_nodes(W_hh1)
    b0 = pad_nodes(b_ih0 + b_hh0)
    b1 = pad_nodes(b_ih1 + b_hh1)

    wlw = np.outer(w_lin[0], w_end[0]).astype(np.float32)  # [H, T]
    cst = np.array([[b_lin[0] * w_end[0].sum() + b_end[0]]], dtype=np.float32)
    ones = np.ones((1, PAIRS * 128), dtype=np.float32)

    in_maps = []
    for c in range(NCORES):
        sl = slice(c * NN, (c + 1) * NN)
        # xTp[t, f, p*128 + s*64 + b] = x[b, t, node, f], node = 16c+2p+s
        xTp = (xp[:, :, sl, :]                 # [B, T, 16, F]
               .transpose(1, 3, 2, 0)          # [T, F, 16, B]
               .reshape(SEQ, FEAT, PAIRS * 128))
        w0 = np.concatenate(
            [Whh0[sl].transpose(2, 0, 1), b0[sl][None],
             Wih0[sl].transpose(2, 0, 1)], axis=0).reshape(K0, NN * 400)
        w1 = np.concatenate(
            [Wih1[sl].transpose(2, 0, 1), b1[sl][None]], axis=0
        ).reshape(H1, NN * 400)
        wh2 = Whh1[sl].transpose(2, 0, 1).reshape(HID, NN * 400)
        bf = ml_dtypes.bfloat16
        in_maps.append({
            "xTp": np.ascontiguousarray(xTp).astype(bf),
            "w0c": np.ascontiguousarray(w0).astype(bf),
            "w1c": np.ascontiguousarray(w1).astype(bf),
            "w1h2": np.ascontiguousarray(wh2).astype(bf),
            "wlw": wlw.astype(bf),
            "cst": cst,
            "ones_d": ones.astype(bf),
        })
    return in_maps


def kernel(x, W_ih0, W_hh0, b_ih0, b_hh0, W_ih1, W_hh1, b_ih1, b_hh1,
           w_lin, b_lin, w_end, b_end):
    global LAST_EXEC_TIME_NS
    args = (x, W_ih0, W_hh0, b_ih0, b_hh0, W_ih1, W_hh1, b_ih1, b_hh1,
            w_lin, b_lin, w_end, b_end)
    args = tuple(np.asarray(a, dtype=np.float32) for a in args)
    in_maps = _host_prep(*args)

    if "prog" not in _PROG_CACHE:
        _PROG_CACHE["prog"] = _build_program()
    nc = _PROG_CACHE["prog"]

    trace = os.environ.get("KERNEL_TRACE", "0") == "1"
    res = run_bass_kernel_spmd(
        nc, in_maps, core_ids=list(range(NCORES)), trace=trace
    )
    LAST_EXEC_TIME_NS = res.exec_time_ns

    out = np.zeros((B, 1, N_NODES, 1), dtype=np.float32)
    hn = np.zeros((N_NODES, 2, B, HID), dtype=np.float32)
    cn = np.zeros((N_NODES, 2, B, HID), dtype=np.float32)
    for c in range(NCORES):
        r = res.results[c]
        nlo, nhi = c * NN, min((c + 1) * NN, N_NODES)
        cnt = nhi - nlo

        # outp: [1, p*128 + s*64 + b] -> [node, b]
        o = r["outp"].reshape(PAIRS, 2, B).reshape(NN, B)
        out[:, 0, nlo:nhi, 0] = o[:cnt].T

        def states(a):  # [128, P*100] -> [node, b, h]
            v = a.reshape(2, B, PAIRS, HID)       # [s, b, p, h]
            return v.transpose(2, 0, 1, 3).reshape(NN, B, HID)

        hn[nlo:nhi, 0] = states(r["h1f"])[:cnt]
        hn[nlo:nhi, 1] = states(r["h2f"])[:cnt]
        cn[nlo:nhi, 0] = states(r["c1f"])[:cnt]
        cn[nlo:nhi, 1] = states(r["c2f"])[:cnt]
    return (out, hn, cn)


# revision 10
# speedup vs baseline: 4.4616x; 1.4089x over previous
"""Trainium2 Bass kernel for nn_AirspaceModel (126 per-node 2-layer LSTMs).

Sharding: 126 nodes padded to 128, 16 nodes per core across 8 cores
(expert-parallel over the independent per-node LSTMs). Each core keeps its
16 nodes' weights resident in SBUF and runs the full T=24 recurrence for
B=64.

Matmul orientation: gates[b, 4H] = xcatT.T @ W^T — the (transposed)
activations are the stationary operand [K<=128, 64] and the per-node
weights stream as rhs [K, 400] in float32r (full-rate 4-byte streaming,
vs 4 cycles/row for plain fp32). Nodes are processed in pairs: the even
node's matmul writes PSUM partitions 0-63 and the odd node's partitions
64-127 (tile_position=(0,64)), so the two matmuls run concurrently in
separate array column groups and elementwise ops see full 128-partition
tiles.

Recurrent states live as [128, pair*101] tiles (batch-pair on partitions).
Each step the new h1/h2 are transposed back to [H, 128] via PE transpose
(h1 carries a built-in ones column so the transpose also produces the
bias row used to fold biases into the matmuls). The transposed h1 of step
t is reused as layer-0 stationary input of step t+1.

The linear+conv head is a weighted sum over (t, h) of layer-2 outputs;
transposed h2 is streamed to DRAM each step and a post-loop phase
accumulates out[n,b] with PSUM-accumulating matmuls against
host-precomputed w_lin[h]*w_end[t] columns.
"""

import os
import sys

import numpy as np
import ml_dtypes

for _p in ("/opt/trn_rl_repo", "/root/.axon_site/_ro/trn_rl_repo"):
    if os.path.isdir(_p) and _p not in sys.path:
        sys.path.append(_p)

import concourse.bass as bass
import concourse.mybir as mybir
import concourse.tile as tile
from concourse import bacc
from concourse.bass_utils import run_bass_kernel_spmd
from concourse.masks import make_identity

F32 = mybir.dt.float32
BF16 = mybir.dt.bfloat16
AF = mybir.ActivationFunctionType

N_NODES, SEQ, FEAT, HID, B = 126, 24, 17, 100, 64
NCORES = 8
NP = 128             # padded node count
NN = NP // NCORES    # nodes per core (16)
PAIRS = NN // 2      # node pairs per core (8)
GP = 4               # pairs per pipeline group
NGRP = PAIRS // GP   # groups (2)
H1 = HID + 1         # h rows + ones col/row (101)
K0 = H1 + FEAT       # layer-0 contraction: h1 + ones + x = 118

LAST_EXEC_TIME_NS = None
_PROG_CACHE = {}


def _build_program():
    nc = bacc.Bacc("TRN2", target_bir_lowering=False, debug=False)

    xTp = nc.dram_tensor("xTp", [SEQ, FEAT, PAIRS * 128], BF16, kind="ExternalInput")
    w0c = nc.dram_tensor("w0c", [K0, NN * 400], BF16, kind="ExternalInput")
    w1c = nc.dram_tensor("w1c", [H1, NN * 400], BF16, kind="ExternalInput")
    w1h2 = nc.dram_tensor("w1h2", [HID, NN * 400], BF16, kind="ExternalInput")
    wlw = nc.dram_tensor("wlw", [HID, SEQ], BF16, kind="ExternalInput")
    cst = nc.dram_tensor("cst", [1, 1], F32, kind="ExternalInput")
    ones_d = nc.dram_tensor("ones_d", [1, PAIRS * 128], BF16, kind="ExternalInput")

    outp = nc.dram_tensor("outp", [1, PAIRS * 128], F32, kind="ExternalOutput")
    h1f = nc.dram_tensor("h1f", [128, PAIRS * HID], F32, kind="ExternalOutput")
    h2f = nc.dram_tensor("h2f", [128, PAIRS * HID], F32, kind="ExternalOutput")
    c1f = nc.dram_tensor("c1f", [128, PAIRS * HID], F32, kind="ExternalOutput")
    c2f = nc.dram_tensor("c2f", [128, PAIRS * HID], F32, kind="ExternalOutput")
    h2allT = nc.dram_tensor(
        "h2allT", [SEQ, HID, PAIRS * 128], BF16, kind="ExternalOutput")

    with tile.TileContext(nc) as tc:
        with (
            tc.tile_pool(name="wpool", bufs=1) as wpool,
            tc.tile_pool(name="state", bufs=1) as state,
            tc.tile_pool(name="xc", bufs=3) as xc,
            tc.tile_pool(name="ew", bufs=3) as ew,
        ):
            # --- resident weights / constants ----------------------------
            w0c_sb = wpool.tile([K0, NN * 400], BF16)
            w1c_sb = wpool.tile([H1, NN * 400], BF16)
            w1h2_sb = wpool.tile([HID, NN * 400], BF16)
            wlw_sb = wpool.tile([HID, SEQ], BF16)
            cst_sb = wpool.tile([1, 1], F32)
            ident = wpool.tile([128, 128], F32)
            nc.sync.dma_start(w0c_sb[:], w0c[:])
            nc.sync.dma_start(w1c_sb[:], w1c[:])
            nc.sync.dma_start(w1h2_sb[:], w1h2[:])
            nc.sync.dma_start(wlw_sb[:], wlw[:])
            nc.sync.dma_start(cst_sb[:], cst[:])
            make_identity(nc, ident[:])

            # --- persistent state (batch-pair on partitions) -------------
            # h1p: [128, 8*101], pair p cols p*101..p*101+100 (col 100=ones)
            h1p = state.tile([128, PAIRS * H1], F32)
            h2p = state.tile([128, PAIRS * HID], F32)
            c1p = state.tile([128, PAIRS * HID], F32)
            c2p = state.tile([128, PAIRS * HID], F32)
            nc.vector.memset(h1p[:], 0.0)
            nc.vector.memset(
                h1p.rearrange("p (n c) -> p n c", c=H1)[:, :, HID : H1], 1.0)
            nc.vector.memset(h2p[:], 0.0)
            nc.vector.memset(c1p[:], 0.0)
            nc.vector.memset(c2p[:], 0.0)

            # transposed stationary inputs, one [K, 8, 128] tile per step:
            # xh1 rows 0-99 h1T, 100 ones, 101-117 x_t; xh2 = h2T
            xh1_prev = xc.tile([K0, PAIRS, 128], BF16, tag="xh1", name="xh1_init")
            nc.vector.memset(xh1_prev[:], 0.0)
            nc.sync.dma_start(
                xh1_prev[HID : H1].rearrange("o p b -> o (p b)"), ones_d[:])
            nc.sync.dma_start(
                xh1_prev[H1:K0].rearrange("o p b -> o (p b)"), xTp[0])
            xh2_prev = xc.tile([HID, PAIRS, 128], BF16, tag="xh2", name="xh2_init")
            nc.vector.memset(xh2_prev[:], 0.0)

            def ewise(gl, hview, cview, sfx):
                """LSTM elementwise for a 4-pair group.

                gl: PSUM gates [128, 4, 512] (cols 0-399 used per pair).
                hview/cview: [128, 4, 100] h destination / cell-state home.
                """
                sf = ew.tile([128, GP * 200], F32, tag="sf" + sfx)
                sf3 = sf.rearrange("p (n c) -> p n c", n=GP)
                tg = ew.tile([128, GP * 100], F32, tag="tg" + sfx)
                tg3 = tg.rearrange("p (n c) -> p n c", n=GP)
                so = ew.tile([128, GP * 100], F32, tag="so" + sfx)
                so3 = so.rearrange("p (n c) -> p n c", n=GP)
                nc.scalar.activation(sf3[:], gl[:, :, 0:200], AF.Sigmoid)
                nc.scalar.activation(tg3[:], gl[:, :, 200:300], AF.Tanh)
                nc.scalar.activation(so3[:], gl[:, :, 300:400], AF.Sigmoid)
                tmp1 = ew.tile([128, GP * 100], F32, tag="t1" + sfx)
                t13 = tmp1.rearrange("p (n c) -> p n c", n=GP)
                tmp2 = ew.tile([128, GP * 100], F32, tag="t2" + sfx)
                t23 = tmp2.rearrange("p (n c) -> p n c", n=GP)
                tcn = ew.tile([128, GP * 100], F32, tag="tc" + sfx)
                tc3 = tcn.rearrange("p (n c) -> p n c", n=GP)
                # c_new = sig(f)*c + sig(i)*tanh(g); the two products run
                # concurrently on different engines
                nc.vector.tensor_mul(t13[:], sf3[:, :, 100:200], cview)
                nc.gpsimd.tensor_mul(t23[:], sf3[:, :, 0:100], tg3[:])
                nc.vector.tensor_add(cview, t13[:], t23[:])
                nc.scalar.activation(tc3[:], cview, AF.Tanh)
                # h = sig(o)*tanh(c_new)
                nc.vector.tensor_mul(hview, so3[:], tc3[:])

            h1p3 = h1p.rearrange("p (n c) -> p n c", c=H1)
            h2p3 = h2p.rearrange("p (n c) -> p n c", c=HID)
            c1p3 = c1p.rearrange("p (n c) -> p n c", c=HID)
            c2p3 = c2p.rearrange("p (n c) -> p n c", c=HID)

            with tc.tile_pool(name="gates", bufs=2, space="PSUM") as gpsum:
                for t in range(SEQ):
                    xh1_new = xc.tile([K0, PAIRS, 128], BF16, tag="xh1",
                                      name=f"xh1_{t}")
                    if t + 1 < SEQ:
                        nc.sync.dma_start(
                            xh1_new[H1:K0].rearrange("o p b -> o (p b)"),
                            xTp[t + 1])
                    xh2_new = xc.tile([HID, PAIRS, 128], BF16, tag="xh2",
                                      name=f"xh2_{t}")

                    # phase-interleaved emission: both groups advance
                    # through each phase together so the two PSUM slots
                    # double-buffer across groups instead of serializing
                    # --- layer 0 matmuls ---------------------------------
                    gl0 = []
                    for g in range(NGRP):
                        gl = gpsum.tile([128, GP, 512], F32, tag="g")
                        gl0.append(gl)
                        for j, p in enumerate(range(g * GP, (g + 1) * GP)):
                            for s in range(2):
                                n = 2 * p + s
                                nc.tensor.matmul(
                                    gl[s * 64 : (s + 1) * 64, j, 0:400],
                                    xh1_prev[0:K0, p, s * 64 : (s + 1) * 64],
                                    w0c_sb[:, n * 400 : (n + 1) * 400],
                                    start=True, stop=True,
                                    tile_position=(0, s * 64),
                                )
                    # --- layer 0 elementwise -----------------------------
                    for g in range(NGRP):
                        ewise(
                            gl0[g],
                            h1p3[:, g * GP : (g + 1) * GP, 0:HID],
                            c1p3[:, g * GP : (g + 1) * GP, :],
                            "a",
                        )
                    # --- transpose new h1 (+ones) for L1 & next L0 -------
                    tpa = []
                    for g in range(NGRP):
                        tp = gpsum.tile([128, GP, 512], F32, tag="g")
                        tpa.append(tp)
                        for j, p in enumerate(range(g * GP, (g + 1) * GP)):
                            nc.tensor.transpose(
                                tp[0:H1, j, 0:128],
                                h1p[:, p * H1 : (p + 1) * H1],
                                ident[:],
                            )
                    for g in range(NGRP):
                        nc.vector.tensor_copy(
                            xh1_new[0:H1, g * GP : (g + 1) * GP, :],
                            tpa[g][0:H1, 0:GP, 0:128],
                        )
                    # --- layer 1 matmuls ---------------------------------
                    gl1 = []
                    for g in range(NGRP):
                        gl = gpsum.tile([128, GP, 512], F32, tag="g")
                        gl1.append(gl)
                        for j, p in enumerate(range(g * GP, (g + 1) * GP)):
                            for s in range(2):
                                n = 2 * p + s
                                sl = slice(s * 64, (s + 1) * 64)
                                nc.tensor.matmul(
                                    gl[sl, j, 0:400],
                                    xh1_new[0:H1, p, sl],
                                    w1c_sb[:, n * 400 : (n + 1) * 400],
                                    start=True, stop=False,
                                    tile_position=(0, s * 64),
                                )
                                nc.tensor.matmul(
                                    gl[sl, j, 0:400],
                                    xh2_prev[0:HID, p, sl],
                                    w1h2_sb[:, n * 400 : (n + 1) * 400],
                                    start=False, stop=True,
                                    tile_position=(0, s * 64),
                                )
                    # --- layer 1 elementwise -----------------------------
                    for g in range(NGRP):
                        ewise(
                            gl1[g],
                            h2p3[:, g * GP : (g + 1) * GP, :],
                            c2p3[:, g * GP : (g + 1) * GP, :],
                            "b",
                        )
                    # --- transpose new h2 --------------------------------
                    tpb = []
                    for g in range(NGRP):
                        tp = gpsum.tile([128, GP, 512], F32, tag="g")
                        tpb.append(tp)
                        for j, p in enumerate(range(g * GP, (g + 1) * GP)):
                            nc.tensor.transpose(
                                tp[0:HID, j, 0:128],
                                h2p[:, p * HID : (p + 1) * HID],
                                ident[:],
                            )
                    for g in range(NGRP):
                        nc.vector.tensor_copy(
                            xh2_new[0:HID, g * GP : (g + 1) * GP, :],
                            tpb[g][0:HID, 0:GP, 0:128],
                        )
                    # stream h2T of this step to DRAM for the head
                    nc.sync.dma_start(
                        h2allT[t], xh2_new.rearrange("o p b -> o (p b)"))
                    xh1_prev = xh1_new
                    xh2_prev = xh2_new

            # --- head: out = sum_t wlw[:,t] . h2allT[t] + cst ------------
            with tc.tile_pool(name="hps", bufs=1, space="PSUM") as hps:
                hp = hps.tile([1, PAIRS * 128], F32)
                for t in range(SEQ):
                    h2c = ew.tile([HID, PAIRS * 128], BF16, tag="h2c")
                    nc.sync.dma_start(h2c[:], h2allT[t])
                    for half in range(2):
                        nc.tensor.matmul(
                            hp[:, half * 512 : (half + 1) * 512],
                            wlw_sb[:, t : t + 1],
                            h2c[:, half * 512 : (half + 1) * 512],
                            start=(t == 0),
                            stop=(t == SEQ - 1),
                        )
                out_sb = ew.tile([1, PAIRS * 128], F32, tag="out_sb")
                nc.scalar.activation(
                    out_sb[:], hp[:], AF.Identity, bias=cst_sb[0:1, 0:1])
                nc.sync.dma_start(outp[:], out_sb[:])

            # --- final states --------------------------------------------
            nc.sync.dma_start(h1f[:], h1p3[:, :, 0:HID])
            nc.sync.dma_start(h2f[:], h2p[:])
            nc.sync.dma_start(c1f[:], c1p[:])
            nc.sync.dma_start(c2f[:], c2p[:])

    nc.compile()
    return nc


def _host_prep(x, W_ih0, W_hh0, b_ih0, b_hh0, W_ih1, W_hh1, b_ih1, b_hh1,
               w_lin, b_lin, w_end, b_end):
    """Pad to 128 nodes and build per-core input maps."""
    def pad_nodes(a):
        pad = [(0, 0)] * a.ndim
        pad[0] = (0, NP - N_NODES)
        return np.pad(a, pad)

    xp = np.pad(x, [(0, 0), (0, 0), (0, NP - N_NODES), (0, 0)])  # [B,T,NP,F]
    Wih0, Whh0 = pad_nodes(W_ih0), pad_nodes(W_hh0)
    Wih1, Whh1 = pad_nodes(W_ih1), pad_nodes(W_hh1)
    b0 = pad_nodes(b_ih0 + b_hh0)
    b1 = pad_nodes(b_ih1 + b_hh1)

    wlw = np.outer(w_lin[0], w_end[0]).astype(np.float32)  # [H, T]
    cst = np.array([[b_lin[0] * w_end[0].sum() + b_end[0]]], dtype=np.float32)
    ones = np.ones((1, PAIRS * 128), dtype=np.float32)

    in_maps = []
    for c in range(NCORES):
        sl = slice(c * NN, (c + 1) * NN)
        # xTp[t, f, p*128 + s*64 + b] = x[b, t, node, f], node = 16c+2p+s
        xTp = (xp[:, :, sl, :]                 # [B, T, 16, F]
               .transpose(1, 3, 2, 0)          # [T, F, 16, B]
               .reshape(SEQ, FEAT, PAIRS * 128))
        w0 = np.concatenate(
            [Whh0[sl].transpose(2, 0, 1), b0[sl][None],
             Wih0[sl].transpose(2, 0, 1)], axis=0).reshape(K0, NN * 400)
        w1 = np.concatenate(
            [Wih1[sl].transpose(2, 0, 1), b1[sl][None]], axis=0
        ).reshape(H1, NN * 400)
        wh2 = Whh1[sl].transpose(2, 0, 1).reshape(HID, NN * 400)
        bf = ml_dtypes.bfloat16
        in_maps.append({
            "xTp": np.ascontiguousarray(xTp).astype(bf),
            "w0c": np.ascontiguousarray(w0).astype(bf),
            "w1c": np.ascontiguousarray(w1).astype(bf),
            "w1h2": np.ascontiguousarray(wh2).astype(bf),
            "wlw": wlw.astype(bf),
            "cst": cst,
            "ones_d": ones.astype(bf),
        })
    return in_maps


def kernel(x, W_ih0, W_hh0, b_ih0, b_hh0, W_ih1, W_hh1, b_ih1, b_hh1,
           w_lin, b_lin, w_end, b_end):
    global LAST_EXEC_TIME_NS
    args = (x, W_ih0, W_hh0, b_ih0, b_hh0, W_ih1, W_hh1, b_ih1, b_hh1,
            w_lin, b_lin, w_end, b_end)
    args = tuple(np.asarray(a, dtype=np.float32) for a in args)
    in_maps = _host_prep(*args)

    if "prog" not in _PROG_CACHE:
        _PROG_CACHE["prog"] = _build_program()
    nc = _PROG_CACHE["prog"]

    trace = os.environ.get("KERNEL_TRACE", "0") == "1"
    res = run_bass_kernel_spmd(
        nc, in_maps, core_ids=list(range(NCORES)), trace=trace
    )
    LAST_EXEC_TIME_NS = res.exec_time_ns

    out = np.zeros((B, 1, N_NODES, 1), dtype=np.float32)
    hn = np.zeros((N_NODES, 2, B, HID), dtype=np.float32)
    cn = np.zeros((N_NODES, 2, B, HID), dtype=np.float32)
    for c in range(NCORES):
        r = res.results[c]
        nlo, nhi = c * NN, min((c + 1) * NN, N_NODES)
        cnt = nhi - nlo

        # outp: [1, p*128 + s*64 + b] -> [node, b]
        o = r["outp"].reshape(PAIRS, 2, B).reshape(NN, B)
        out[:, 0, nlo:nhi, 0] = o[:cnt].T

        def states(a):  # [128, P*100] -> [node, b, h]
            v = a.reshape(2, B, PAIRS, HID)       # [s, b, p, h]
            return v.transpose(2, 0, 1, 3).reshape(NN, B, HID)

        hn[nlo:nhi, 0] = states(r["h1f"])[:cnt]
        hn[nlo:nhi, 1] = states(r["h2f"])[:cnt]
        cn[nlo:nhi, 0] = states(r["c1f"])[:cnt]
        cn[nlo:nhi, 1] = states(r["c2f"])[:cnt]
    return (out, hn, cn)


# revision 13
# speedup vs baseline: 5.1287x; 1.1495x over previous
"""Trainium2 Bass kernel for nn_AirspaceModel (126 per-node 2-layer LSTMs).

Sharding: 126 nodes padded to 128, 16 nodes per core across 8 cores
(expert-parallel over the independent per-node LSTMs). Each core keeps its
16 nodes' weights resident in SBUF and runs the full T=24 recurrence for
B=64.

Matmul orientation: gates[b, 4H] = xcatT.T @ W^T — the (transposed)
activations are the stationary operand [K<=128, 64] and the per-node
weights stream as rhs [K, 400] in float32r (full-rate 4-byte streaming,
vs 4 cycles/row for plain fp32). Nodes are processed in pairs: the even
node's matmul writes PSUM partitions 0-63 and the odd node's partitions
64-127 (tile_position=(0,64)), so the two matmuls run concurrently in
separate array column groups and elementwise ops see full 128-partition
tiles.

Recurrent states live as [128, pair*101] tiles (batch-pair on partitions).
Each step the new h1/h2 are transposed back to [H, 128] via PE transpose
(h1 carries a built-in ones column so the transpose also produces the
bias row used to fold biases into the matmuls). The transposed h1 of step
t is reused as layer-0 stationary input of step t+1.

The linear+conv head is a weighted sum over (t, h) of layer-2 outputs;
transposed h2 is streamed to DRAM each step and a post-loop phase
accumulates out[n,b] with PSUM-accumulating matmuls against
host-precomputed w_lin[h]*w_end[t] columns.
"""

import os
import sys

import numpy as np
import ml_dtypes

for _p in ("/opt/trn_rl_repo", "/root/.axon_site/_ro/trn_rl_repo"):
    if os.path.isdir(_p) and _p not in sys.path:
        sys.path.append(_p)

import concourse.bass as bass
import concourse.mybir as mybir
import concourse.tile as tile
from concourse import bacc
from concourse.bass_utils import run_bass_kernel_spmd
from concourse.masks import make_identity

F32 = mybir.dt.float32
BF16 = mybir.dt.bfloat16
AF = mybir.ActivationFunctionType

N_NODES, SEQ, FEAT, HID, B = 126, 24, 17, 100, 64
NCORES = 8
NP = 128             # padded node count
NN = NP // NCORES    # nodes per core (16)
PAIRS = NN // 2      # node pairs per core (8)
GP = 4               # pairs per pipeline group
NGRP = PAIRS // GP   # groups (2)
H1 = HID + 1         # h rows + ones col/row (101)
K0 = H1 + FEAT       # layer-0 contraction: h1 + ones + x = 118

LAST_EXEC_TIME_NS = None
_PROG_CACHE = {}


def _build_program():
    nc = bacc.Bacc("TRN2", target_bir_lowering=False, debug=False)

    xTp = nc.dram_tensor("xTp", [SEQ, FEAT, PAIRS * 128], BF16, kind="ExternalInput")
    w0c = nc.dram_tensor("w0c", [K0, NN * 400], BF16, kind="ExternalInput")
    w1c = nc.dram_tensor("w1c", [H1, NN * 400], BF16, kind="ExternalInput")
    w1h2 = nc.dram_tensor("w1h2", [HID, NN * 400], BF16, kind="ExternalInput")
    wlw = nc.dram_tensor("wlw", [HID, SEQ], BF16, kind="ExternalInput")
    cst = nc.dram_tensor("cst", [1, 1], F32, kind="ExternalInput")
    ones_d = nc.dram_tensor("ones_d", [1, PAIRS * 128], BF16, kind="ExternalInput")

    outp = nc.dram_tensor("outp", [1, PAIRS * 128], F32, kind="ExternalOutput")
    h1f = nc.dram_tensor("h1f", [128, PAIRS * HID], F32, kind="ExternalOutput")
    h2f = nc.dram_tensor("h2f", [128, PAIRS * HID], F32, kind="ExternalOutput")
    c1f = nc.dram_tensor("c1f", [128, PAIRS * HID], F32, kind="ExternalOutput")
    c2f = nc.dram_tensor("c2f", [128, PAIRS * HID], F32, kind="ExternalOutput")
    h2allT = nc.dram_tensor(
        "h2allT", [SEQ, HID, PAIRS * 128], BF16, kind="ExternalOutput")

    with tile.TileContext(nc) as tc:
        with (
            tc.tile_pool(name="wpool", bufs=1) as wpool,
            tc.tile_pool(name="state", bufs=1) as state,
            tc.tile_pool(name="xc", bufs=3) as xc,
            tc.tile_pool(name="ew", bufs=3) as ew,
        ):
            # --- resident weights / constants ----------------------------
            w0c_sb = wpool.tile([K0, NN * 400], BF16)
            w1c_sb = wpool.tile([H1, NN * 400], BF16)
            w1h2_sb = wpool.tile([HID, NN * 400], BF16)
            wlw_sb = wpool.tile([HID, SEQ], BF16)
            cst_sb = wpool.tile([1, 1], F32)
            ident = wpool.tile([128, 128], F32)
            # chunked weight loads: the first group's nodes arrive first so
            # layer-0 matmuls of step 0 don't wait for the full 4MB
            for lo in range(0, NN * 400, 4 * 400):
                hi = lo + 4 * 400
                nc.sync.dma_start(w0c_sb[:, lo:hi], w0c[:, lo:hi])
            for lo in range(0, NN * 400, 4 * 400):
                hi = lo + 4 * 400
                nc.sync.dma_start(w1c_sb[:, lo:hi], w1c[:, lo:hi])
                nc.sync.dma_start(w1h2_sb[:, lo:hi], w1h2[:, lo:hi])
            nc.sync.dma_start(wlw_sb[:], wlw[:])
            nc.sync.dma_start(cst_sb[:], cst[:])
            make_identity(nc, ident[:])

            # --- persistent state (batch-pair on partitions) -------------
            # h1p: [128, 8*101], pair p cols p*101..p*101+100 (col 100=ones)
            h1p = state.tile([128, PAIRS * H1], F32)
            h2p = state.tile([128, PAIRS * HID], F32)
            c1p = state.tile([128, PAIRS * HID], F32)
            c2p = state.tile([128, PAIRS * HID], F32)
            nc.vector.memset(h1p[:], 0.0)
            nc.vector.memset(
                h1p.rearrange("p (n c) -> p n c", c=H1)[:, :, HID : H1], 1.0)
            nc.vector.memset(h2p[:], 0.0)
            nc.vector.memset(c1p[:], 0.0)
            nc.vector.memset(c2p[:], 0.0)

            # transposed stationary inputs, one [K, 8, 128] tile per step:
            # xh1 rows 0-99 h1T, 100 ones, 101-117 x_t; xh2 = h2T
            xh1_prev = xc.tile([K0, PAIRS, 128], BF16, tag="xh1", name="xh1_init")
            nc.vector.memset(xh1_prev[:], 0.0)
            nc.sync.dma_start(
                xh1_prev[HID : H1].rearrange("o p b -> o (p b)"), ones_d[:])
            nc.sync.dma_start(
                xh1_prev[H1:K0].rearrange("o p b -> o (p b)"), xTp[0])

            def ewise(gl, hview, cview, sfx):
                """LSTM elementwise for a 4-pair group.

                gl: PSUM gates [128, 4, 512] (cols 0-399 used per pair).
                hview/cview: [128, 4, 100] h destination / cell-state home.
                """
                sf = ew.tile([128, GP * 200], F32, tag="sf" + sfx)
                sf3 = sf.rearrange("p (n c) -> p n c", n=GP)
                tg = ew.tile([128, GP * 100], F32, tag="tg" + sfx)
                tg3 = tg.rearrange("p (n c) -> p n c", n=GP)
                so = ew.tile([128, GP * 100], F32, tag="so" + sfx)
                so3 = so.rearrange("p (n c) -> p n c", n=GP)
                nc.scalar.activation(sf3[:], gl[:, :, 0:200], AF.Sigmoid)
                nc.scalar.activation(tg3[:], gl[:, :, 200:300], AF.Tanh)
                nc.scalar.activation(so3[:], gl[:, :, 300:400], AF.Sigmoid)
                tmp1 = ew.tile([128, GP * 100], F32, tag="t1" + sfx)
                t13 = tmp1.rearrange("p (n c) -> p n c", n=GP)
                tmp2 = ew.tile([128, GP * 100], F32, tag="t2" + sfx)
                t23 = tmp2.rearrange("p (n c) -> p n c", n=GP)
                tcn = ew.tile([128, GP * 100], F32, tag="tc" + sfx)
                tc3 = tcn.rearrange("p (n c) -> p n c", n=GP)
                # c_new = sig(f)*c + sig(i)*tanh(g); the two products run
                # concurrently on different engines
                nc.vector.tensor_mul(t13[:], sf3[:, :, 100:200], cview)
                nc.gpsimd.tensor_mul(t23[:], sf3[:, :, 0:100], tg3[:])
                nc.vector.tensor_add(cview, t13[:], t23[:])
                nc.scalar.activation(tc3[:], cview, AF.Tanh)
                # h = sig(o)*tanh(c_new)
                nc.vector.tensor_mul(hview, so3[:], tc3[:])

            h1p3 = h1p.rearrange("p (n c) -> p n c", c=H1)
            h2p3 = h2p.rearrange("p (n c) -> p n c", c=HID)
            c1p3 = c1p.rearrange("p (n c) -> p n c", c=HID)
            c2p3 = c2p.rearrange("p (n c) -> p n c", c=HID)

            with tc.tile_pool(name="gates", bufs=2, space="PSUM") as gpsum:

                def transpose_h2(xh2_cur, t_src):
                    """Transpose h2p (holding h2(t_src)) into xh2_cur and
                    stream it to h2allT[t_src]."""
                    for g in range(NGRP):
                        tp = gpsum.tile([128, GP, 512], F32, tag="g",
                                        name="tpb")
                        for j, p in enumerate(range(g * GP, (g + 1) * GP)):
                            nc.tensor.transpose(
                                tp[0:HID, j, 0:128],
                                h2p[:, p * HID : (p + 1) * HID],
                                ident[:],
                            )
                        nc.vector.tensor_copy(
                            xh2_cur[0:HID, g * GP : (g + 1) * GP, :],
                            tp[0:HID, 0:GP, 0:128],
                        )
                    nc.sync.dma_start(
                        h2allT[t_src], xh2_cur.rearrange("o p b -> o (p b)"))

                # software-pipelined emission: iteration t emits the h2
                # transpose of step t-1 between this step's layer-0 matmuls
                # and layer-0 elementwise, so every PSUM slot wait lands on
                # the natural predecessor and the PE never stalls a full
                # elementwise chain
                for t in range(SEQ):
                    xh1_new = xc.tile([K0, PAIRS, 128], BF16, tag="xh1",
                                      name=f"xh1_{t}")
                    if t + 1 < SEQ:
                        nc.sync.dma_start(
                            xh1_new[H1:K0].rearrange("o p b -> o (p b)"),
                            xTp[t + 1])
                    xh2_cur = xc.tile([HID, PAIRS, 128], BF16, tag="xh2",
                                      name=f"xh2_{t}")

                    # --- layer 0 matmuls (h1T(t-1) + x(t)) ---------------
                    gl0 = []
                    for g in range(NGRP):
                        gl = gpsum.tile([128, GP, 512], F32, tag="g",
                                        name="gl0")
                        gl0.append(gl)
                        for j, p in enumerate(range(g * GP, (g + 1) * GP)):
                            for s in range(2):
                                n = 2 * p + s
                                nc.tensor.matmul(
                                    gl[s * 64 : (s + 1) * 64, j, 0:400],
                                    xh1_prev[0:K0, p, s * 64 : (s + 1) * 64],
                                    w0c_sb[:, n * 400 : (n + 1) * 400],
                                    start=True, stop=True,
                                    tile_position=(0, s * 64),
                                )
                    # --- h2T of the previous step ------------------------
                    if t == 0:
                        nc.vector.memset(xh2_cur[:], 0.0)
                    else:
                        transpose_h2(xh2_cur, t - 1)
                    # --- layer 0 elementwise -----------------------------
                    for g in range(NGRP):
                        ewise(
                            gl0[g],
                            h1p3[:, g * GP : (g + 1) * GP, 0:HID],
                            c1p3[:, g * GP : (g + 1) * GP, :],
                            "a",
                        )
                    # --- transpose new h1 (+ones) for L1 & next L0 -------
                    tpa = []
                    for g in range(NGRP):
                        tp = gpsum.tile([128, GP, 512], F32, tag="g",
                                        name="tpa")
                        tpa.append(tp)
                        for j, p in enumerate(range(g * GP, (g + 1) * GP)):
                            nc.tensor.transpose(
                                tp[0:H1, j, 0:128],
                                h1p[:, p * H1 : (p + 1) * H1],
                                ident[:],
                            )
                    for g in range(NGRP):
                        nc.vector.tensor_copy(
                            xh1_new[0:H1, g * GP : (g + 1) * GP, :],
                            tpa[g][0:H1, 0:GP, 0:128],
                        )
                    # --- layer 1 matmuls ---------------------------------
                    gl1 = []
                    for g in range(NGRP):
                        gl = gpsum.tile([128, GP, 512], F32, tag="g",
                                        name="gl1")
                        gl1.append(gl)
                        for j, p in enumerate(range(g * GP, (g + 1) * GP)):
                            for s in range(2):
                                n = 2 * p + s
                                sl = slice(s * 64, (s + 1) * 64)
                                nc.tensor.matmul(
                                    gl[sl, j, 0:400],
                                    xh1_new[0:H1, p, sl],
                                    w1c_sb[:, n * 400 : (n + 1) * 400],
                                    start=True, stop=False,
                                    tile_position=(0, s * 64),
                                )
                                nc.tensor.matmul(
                                    gl[sl, j, 0:400],
                                    xh2_cur[0:HID, p, sl],
                                    w1h2_sb[:, n * 400 : (n + 1) * 400],
                                    start=False, stop=True,
                                    tile_position=(0, s * 64),
                                )
                    # --- layer 1 elementwise -----------------------------
                    for g in range(NGRP):
                        ewise(
                            gl1[g],
                            h2p3[:, g * GP : (g + 1) * GP, :],
                            c2p3[:, g * GP : (g + 1) * GP, :],
                            "b",
                        )
                    xh1_prev = xh1_new

                # final h2 transpose for the head (t = SEQ-1)
                xh2_last = xc.tile([HID, PAIRS, 128], BF16, tag="xh2",
                                   name="xh2_last")
                transpose_h2(xh2_last, SEQ - 1)

            # --- head: out = sum_t wlw[:,t] . h2allT[t] + cst ------------
            with tc.tile_pool(name="hps", bufs=1, space="PSUM") as hps:
                hp = hps.tile([1, PAIRS * 128], F32)
                for t in range(SEQ):
                    h2c = ew.tile([HID, PAIRS * 128], BF16, tag="h2c")
                    nc.sync.dma_start(h2c[:], h2allT[t])
                    for half in range(2):
                        nc.tensor.matmul(
                            hp[:, half * 512 : (half + 1) * 512],
                            wlw_sb[:, t : t + 1],
                            h2c[:, half * 512 : (half + 1) * 512],
                            start=(t == 0),
                            stop=(t == SEQ - 1),
                        )
                out_sb = ew.tile([1, PAIRS * 128], F32, tag="out_sb")
                nc.scalar.activation(
                    out_sb[:], hp[:], AF.Identity, bias=cst_sb[0:1, 0:1])
                nc.sync.dma_start(outp[:], out_sb[:])

            # --- final states --------------------------------------------
            nc.sync.dma_start(h1f[:], h1p3[:, :, 0:HID])
            nc.sync.dma_start(h2f[:], h2p[:])
            nc.sync.dma_start(c1f[:], c1p[:])
            nc.sync.dma_start(c2f[:], c2p[:])

    nc.compile()
    return nc


def _host_prep(x, W_ih0, W_hh0, b_ih0, b_hh0, W_ih1, W_hh1, b_ih1, b_hh1,
               w_lin, b_lin, w_end, b_end):
    """Pad to 128 nodes and build per-core input maps."""
    def pad_nodes(a):
        pad = [(0, 0)] * a.ndim
        pad[0] = (0, NP - N_NODES)
        return np.pad(a, pad)

    xp = np.pad(x, [(0, 0), (0, 0), (0, NP - N_NODES), (0, 0)])  # [B,T,NP,F]
    Wih0, Whh0 = pad_nodes(W_ih0), pad_nodes(W_hh0)
    Wih1, Whh1 = pad_nodes(W_ih1), pad_nodes(W_hh1)
    b0 = pad_nodes(b_ih0 + b_hh0)
    b1 = pad_nodes(b_ih1 + b_hh1)

    wlw = np.outer(w_lin[0], w_end[0]).astype(np.float32)  # [H, T]
    cst = np.array([[b_lin[0] * w_end[0].sum() + b_end[0]]], dtype=np.float32)
    ones = np.ones((1, PAIRS * 128), dtype=np.float32)

    in_maps = []
    for c in range(NCORES):
        sl = slice(c * NN, (c + 1) * NN)
        # xTp[t, f, p*128 + s*64 + b] = x[b, t, node, f], node = 16c+2p+s
        xTp = (xp[:, :, sl, :]                 # [B, T, 16, F]
               .transpose(1, 3, 2, 0)          # [T, F, 16, B]
               .reshape(SEQ, FEAT, PAIRS * 128))
        w0 = np.concatenate(
            [Whh0[sl].transpose(2, 0, 1), b0[sl][None],
             Wih0[sl].transpose(2, 0, 1)], axis=0).reshape(K0, NN * 400)
        w1 = np.concatenate(
            [Wih1[sl].transpose(2, 0, 1), b1[sl][None]], axis=0
        ).reshape(H1, NN * 400)
        wh2 = Whh1[sl].transpose(2, 0, 1).reshape(HID, NN * 400)
        bf = ml_dtypes.bfloat16
        in_maps.append({
            "xTp": np.ascontiguousarray(xTp).astype(bf),
            "w0c": np.ascontiguousarray(w0).astype(bf),
            "w1c": np.ascontiguousarray(w1).astype(bf),
            "w1h2": np.ascontiguousarray(wh2).astype(bf),
            "wlw": wlw.astype(bf),
            "cst": cst,
            "ones_d": ones.astype(bf),
        })
    return in_maps


def kernel(x, W_ih0, W_hh0, b_ih0, b_hh0, W_ih1, W_hh1, b_ih1, b_hh1,
           w_lin, b_lin, w_end, b_end):
    global LAST_EXEC_TIME_NS
    args = (x, W_ih0, W_hh0, b_ih0, b_hh0, W_ih1, W_hh1, b_ih1, b_hh1,
            w_lin, b_lin, w_end, b_end)
    args = tuple(np.asarray(a, dtype=np.float32) for a in args)
    in_maps = _host_prep(*args)

    if "prog" not in _PROG_CACHE:
        _PROG_CACHE["prog"] = _build_program()
    nc = _PROG_CACHE["prog"]

    trace = os.environ.get("KERNEL_TRACE", "0") == "1"
    res = run_bass_kernel_spmd(
        nc, in_maps, core_ids=list(range(NCORES)), trace=trace
    )
    LAST_EXEC_TIME_NS = res.exec_time_ns

    out = np.zeros((B, 1, N_NODES, 1), dtype=np.float32)
    hn = np.zeros((N_NODES, 2, B, HID), dtype=np.float32)
    cn = np.zeros((N_NODES, 2, B, HID), dtype=np.float32)
    for c in range(NCORES):
        r = res.results[c]
        nlo, nhi = c * NN, min((c + 1) * NN, N_NODES)
        cnt = nhi - nlo

        # outp: [1, p*128 + s*64 + b] -> [node, b]
        o = r["outp"].reshape(PAIRS, 2, B).reshape(NN, B)
        out[:, 0, nlo:nhi, 0] = o[:cnt].T

        def states(a):  # [128, P*100] -> [node, b, h]
            v = a.reshape(2, B, PAIRS, HID)       # [s, b, p, h]
            return v.transpose(2, 0, 1, 3).reshape(NN, B, HID)

        hn[nlo:nhi, 0] = states(r["h1f"])[:cnt]
        hn[nlo:nhi, 1] = states(r["h2f"])[:cnt]
        cn[nlo:nhi, 0] = states(r["c1f"])[:cnt]
        cn[nlo:nhi, 1] = states(r["c2f"])[:cnt]
    return (out, hn, cn)


# revision 14
# speedup vs baseline: 5.1699x; 1.0080x over previous
"""Trainium2 Bass kernel for nn_AirspaceModel (126 per-node 2-layer LSTMs).

Sharding: 126 nodes padded to 128, 16 nodes per core across 8 cores
(expert-parallel over the independent per-node LSTMs). Each core keeps its
16 nodes' weights resident in SBUF and runs the full T=24 recurrence for
B=64.

Matmul orientation: gates[b, 4H] = xcatT.T @ W^T — the (transposed)
activations are the stationary operand [K<=128, 64] and the per-node
weights stream as rhs [K, 400] in float32r (full-rate 4-byte streaming,
vs 4 cycles/row for plain fp32). Nodes are processed in pairs: the even
node's matmul writes PSUM partitions 0-63 and the odd node's partitions
64-127 (tile_position=(0,64)), so the two matmuls run concurrently in
separate array column groups and elementwise ops see full 128-partition
tiles.

Recurrent states live as [128, pair*101] tiles (batch-pair on partitions).
Each step the new h1/h2 are transposed back to [H, 128] via PE transpose
(h1 carries a built-in ones column so the transpose also produces the
bias row used to fold biases into the matmuls). The transposed h1 of step
t is reused as layer-0 stationary input of step t+1.

The linear+conv head is a weighted sum over (t, h) of layer-2 outputs;
transposed h2 is streamed to DRAM each step and a post-loop phase
accumulates out[n,b] with PSUM-accumulating matmuls against
host-precomputed w_lin[h]*w_end[t] columns.
"""

import os
import sys

import numpy as np
import ml_dtypes

for _p in ("/opt/trn_rl_repo", "/root/.axon_site/_ro/trn_rl_repo"):
    if os.path.isdir(_p) and _p not in sys.path:
        sys.path.append(_p)

import concourse.bass as bass
import concourse.mybir as mybir
import concourse.tile as tile
from concourse import bacc
from concourse.bass_utils import run_bass_kernel_spmd
from concourse.masks import make_identity

F32 = mybir.dt.float32
BF16 = mybir.dt.bfloat16
AF = mybir.ActivationFunctionType

N_NODES, SEQ, FEAT, HID, B = 126, 24, 17, 100, 64
NCORES = 8
NP = 128             # padded node count
NN = NP // NCORES    # nodes per core (16)
PAIRS = NN // 2      # node pairs per core (8)
GP = 4               # pairs per pipeline group
NGRP = PAIRS // GP   # groups (2)
H1 = HID + 1         # h rows + ones col/row (101)
K0 = H1 + FEAT       # layer-0 contraction: h1 + ones + x = 118

LAST_EXEC_TIME_NS = None
_PROG_CACHE = {}


def _build_program():
    nc = bacc.Bacc("TRN2", target_bir_lowering=False, debug=False)

    xTp = nc.dram_tensor("xTp", [SEQ, FEAT, PAIRS * 128], BF16, kind="ExternalInput")
    w0c = nc.dram_tensor("w0c", [K0, NN * 400], BF16, kind="ExternalInput")
    w1c = nc.dram_tensor("w1c", [H1, NN * 400], BF16, kind="ExternalInput")
    w1h2 = nc.dram_tensor("w1h2", [HID, NN * 400], BF16, kind="ExternalInput")
    wlw = nc.dram_tensor("wlw", [HID, SEQ], BF16, kind="ExternalInput")
    cst = nc.dram_tensor("cst", [1, 1], F32, kind="ExternalInput")
    ones_d = nc.dram_tensor("ones_d", [1, PAIRS * 128], BF16, kind="ExternalInput")

    outp = nc.dram_tensor("outp", [1, PAIRS * 128], F32, kind="ExternalOutput")
    h1f = nc.dram_tensor("h1f", [128, PAIRS * HID], F32, kind="ExternalOutput")
    h2f = nc.dram_tensor("h2f", [128, PAIRS * HID], F32, kind="ExternalOutput")
    c1f = nc.dram_tensor("c1f", [128, PAIRS * HID], F32, kind="ExternalOutput")
    c2f = nc.dram_tensor("c2f", [128, PAIRS * HID], F32, kind="ExternalOutput")

    with tile.TileContext(nc) as tc:
        with (
            tc.tile_pool(name="wpool", bufs=1) as wpool,
            tc.tile_pool(name="state", bufs=1) as state,
            tc.tile_pool(name="xc", bufs=3) as xc,
            tc.tile_pool(name="h2h", bufs=SEQ + 2) as h2h,
            tc.tile_pool(name="ew", bufs=2) as ew,
        ):
            # --- resident weights / constants ----------------------------
            w0c_sb = wpool.tile([K0, NN * 400], BF16)
            w1c_sb = wpool.tile([H1, NN * 400], BF16)
            w1h2_sb = wpool.tile([HID, NN * 400], BF16)
            wlw_sb = wpool.tile([HID, SEQ], BF16)
            cst_sb = wpool.tile([1, 1], F32)
            ident = wpool.tile([128, 128], F32)
            # chunked weight loads spread across the three DMA dispatch
            # engines (sync/scalar HWDGE + gpsimd SWDGE) so they parallelize
            for lo in range(0, NN * 400, 4 * 400):
                hi = lo + 4 * 400
                nc.sync.dma_start(w0c_sb[:, lo:hi], w0c[:, lo:hi])
                nc.scalar.dma_start(w1c_sb[:, lo:hi], w1c[:, lo:hi])
                nc.gpsimd.dma_start(w1h2_sb[:, lo:hi], w1h2[:, lo:hi])
            nc.scalar.dma_start(wlw_sb[:], wlw[:])
            nc.scalar.dma_start(cst_sb[:], cst[:])
            make_identity(nc, ident[:])

            # --- persistent state (batch-pair on partitions) -------------
            # h1p: [128, 8*101], pair p cols p*101..p*101+100 (col 100=ones)
            h1p = state.tile([128, PAIRS * H1], F32)
            h2p = state.tile([128, PAIRS * HID], F32)
            c1p = state.tile([128, PAIRS * HID], F32)
            c2p = state.tile([128, PAIRS * HID], F32)
            nc.vector.memset(h1p[:], 0.0)
            nc.vector.memset(
                h1p.rearrange("p (n c) -> p n c", c=H1)[:, :, HID : H1], 1.0)
            nc.vector.memset(h2p[:], 0.0)
            nc.vector.memset(c1p[:], 0.0)
            nc.vector.memset(c2p[:], 0.0)

            # transposed stationary inputs, one [K, 8, 128] tile per step:
            # xh1 rows 0-99 h1T, 100 ones, 101-117 x_t; xh2 = h2T
            xh1_prev = xc.tile([K0, PAIRS, 128], BF16, tag="xh1", name="xh1_init")
            nc.vector.memset(xh1_prev[:], 0.0)
            nc.sync.dma_start(
                xh1_prev[HID : H1].rearrange("o p b -> o (p b)"), ones_d[:])
            nc.sync.dma_start(
                xh1_prev[H1:K0].rearrange("o p b -> o (p b)"), xTp[0])

            def ewise(gl, hview, cview, sfx):
                """LSTM elementwise for a 4-pair group.

                gl: PSUM gates [128, 4, 512] (cols 0-399 used per pair).
                hview/cview: [128, 4, 100] h destination / cell-state home.
                """
                # gates are host-reordered to [i, f, o, g] so one sigmoid
                # covers i,f,o
                sf = ew.tile([128, GP * 300], F32, tag="sf" + sfx)
                sf3 = sf.rearrange("p (n c) -> p n c", n=GP)
                tg = ew.tile([128, GP * 100], F32, tag="tg" + sfx)
                tg3 = tg.rearrange("p (n c) -> p n c", n=GP)
                nc.scalar.activation(sf3[:], gl[:, :, 0:300], AF.Sigmoid)
                nc.scalar.activation(tg3[:], gl[:, :, 300:400], AF.Tanh)
                tmp1 = ew.tile([128, GP * 100], F32, tag="t1" + sfx)
                t13 = tmp1.rearrange("p (n c) -> p n c", n=GP)
                tmp2 = ew.tile([128, GP * 100], F32, tag="t2" + sfx)
                t23 = tmp2.rearrange("p (n c) -> p n c", n=GP)
                tcn = ew.tile([128, GP * 100], F32, tag="tc" + sfx)
                tc3 = tcn.rearrange("p (n c) -> p n c", n=GP)
                # c_new = sig(f)*c + sig(i)*tanh(g); the two products run
                # concurrently on different engines
                nc.vector.tensor_mul(t13[:], sf3[:, :, 100:200], cview)
                nc.gpsimd.tensor_mul(t23[:], sf3[:, :, 0:100], tg3[:])
                nc.vector.tensor_add(cview, t13[:], t23[:])
                nc.scalar.activation(tc3[:], cview, AF.Tanh)
                # h = sig(o)*tanh(c_new)
                nc.gpsimd.tensor_mul(hview, sf3[:, :, 200:300], tc3[:])

            h1p3 = h1p.rearrange("p (n c) -> p n c", c=H1)
            h2p3 = h2p.rearrange("p (n c) -> p n c", c=HID)
            c1p3 = c1p.rearrange("p (n c) -> p n c", c=HID)
            c2p3 = c2p.rearrange("p (n c) -> p n c", c=HID)

            with tc.tile_pool(name="gates", bufs=2, space="PSUM") as gpsum:

                def transpose_h2(xh2_cur, t_src):
                    """Transpose h2p (holding h2(t_src)) into xh2_cur (kept
                    resident in SBUF for layer-1 and the head)."""
                    for g in range(NGRP):
                        tp = gpsum.tile([128, GP, 512], F32, tag="g",
                                        name="tpb")
                        for j, p in enumerate(range(g * GP, (g + 1) * GP)):
                            nc.tensor.transpose(
                                tp[0:HID, j, 0:128],
                                h2p[:, p * HID : (p + 1) * HID],
                                ident[:],
                            )
                        nc.vector.tensor_copy(
                            xh2_cur[0:HID, g * GP : (g + 1) * GP, :],
                            tp[0:HID, 0:GP, 0:128],
                        )

                h2hist = []
                # software-pipelined emission: iteration t emits the h2
                # transpose of step t-1 between this step's layer-0 matmuls
                # and layer-0 elementwise, so every PSUM slot wait lands on
                # the natural predecessor and the PE never stalls a full
                # elementwise chain
                for t in range(SEQ):
                    xh1_new = xc.tile([K0, PAIRS, 128], BF16, tag="xh1",
                                      name=f"xh1_{t}")
                    if t + 1 < SEQ:
                        nc.sync.dma_start(
                            xh1_new[H1:K0].rearrange("o p b -> o (p b)"),
                            xTp[t + 1])
                    xh2_cur = h2h.tile([HID, PAIRS, 128], BF16, tag="xh2",
                                       name=f"xh2_{t}")
                    h2hist.append(xh2_cur)

                    # --- layer 0 matmuls (h1T(t-1) + x(t)) ---------------
                    gl0 = []
                    for g in range(NGRP):
                        gl = gpsum.tile([128, GP, 512], F32, tag="g",
                                        name="gl0")
                        gl0.append(gl)
                        for j, p in enumerate(range(g * GP, (g + 1) * GP)):
                            for s in range(2):
                                n = 2 * p + s
                                nc.tensor.matmul(
                                    gl[s * 64 : (s + 1) * 64, j, 0:400],
                                    xh1_prev[0:K0, p, s * 64 : (s + 1) * 64],
                                    w0c_sb[:, n * 400 : (n + 1) * 400],
                                    start=True, stop=True,
                                    tile_position=(0, s * 64),
                                )
                    # --- h2T of the previous step ------------------------
                    if t == 0:
                        nc.vector.memset(xh2_cur[:], 0.0)
                    else:
                        transpose_h2(xh2_cur, t - 1)
                    # --- layer 0 elementwise -----------------------------
                    for g in range(NGRP):
                        ewise(
                            gl0[g],
                            h1p3[:, g * GP : (g + 1) * GP, 0:HID],
                            c1p3[:, g * GP : (g + 1) * GP, :],
                            "a",
                        )
                    # --- transpose new h1 (+ones) for L1 & next L0 -------
                    tpa = []
                    for g in range(NGRP):
                        tp = gpsum.tile([128, GP, 512], F32, tag="g",
                                        name="tpa")
                        tpa.append(tp)
                        for j, p in enumerate(range(g * GP, (g + 1) * GP)):
                            nc.tensor.transpose(
                                tp[0:H1, j, 0:128],
                                h1p[:, p * H1 : (p + 1) * H1],
                                ident[:],
                            )
                    for g in range(NGRP):
                        nc.vector.tensor_copy(
                            xh1_new[0:H1, g * GP : (g + 1) * GP, :],
                            tpa[g][0:H1, 0:GP, 0:128],
                        )
                    # --- layer 1 matmuls ---------------------------------
                    gl1 = []
                    for g in range(NGRP):
                        gl = gpsum.tile([128, GP, 512], F32, tag="g",
                                        name="gl1")
                        gl1.append(gl)
                        for j, p in enumerate(range(g * GP, (g + 1) * GP)):
                            for s in range(2):
                                n = 2 * p + s
                                sl = slice(s * 64, (s + 1) * 64)
                                nc.tensor.matmul(
                                    gl[sl, j, 0:400],
                                    xh1_new[0:H1, p, sl],
                                    w1c_sb[:, n * 400 : (n + 1) * 400],
                                    start=True, stop=False,
                                    tile_position=(0, s * 64),
                                )
                                nc.tensor.matmul(
                                    gl[sl, j, 0:400],
                                    xh2_cur[0:HID, p, sl],
                                    w1h2_sb[:, n * 400 : (n + 1) * 400],
                                    start=False, stop=True,
                                    tile_position=(0, s * 64),
                                )
                    # --- layer 1 elementwise -----------------------------
                    for g in range(NGRP):
                        ewise(
                            gl1[g],
                            h2p3[:, g * GP : (g + 1) * GP, :],
                            c2p3[:, g * GP : (g + 1) * GP, :],
                            "b",
                        )
                    xh1_prev = xh1_new

                # final h2 transpose for the head (t = SEQ-1)
                xh2_last = h2h.tile([HID, PAIRS, 128], BF16, tag="xh2",
                                    name="xh2_last")
                transpose_h2(xh2_last, SEQ - 1)
                h2hist.append(xh2_last)

            # --- head: out = sum_t wlw[:,t] . h2allT[t] + cst ------------
            with tc.tile_pool(name="hps", bufs=1, space="PSUM") as hps:
                hp = hps.tile([1, PAIRS * 128], F32)
                for t in range(SEQ):
                    h2c = h2hist[t + 1].rearrange("o p b -> o (p b)")
                    for half in range(2):
                        nc.tensor.matmul(
                            hp[:, half * 512 : (half + 1) * 512],
                            wlw_sb[:, t : t + 1],
                            h2c[:, half * 512 : (half + 1) * 512],
                            start=(t == 0),
                            stop=(t == SEQ - 1),
                        )
                out_sb = ew.tile([1, PAIRS * 128], F32, tag="out_sb")
                nc.scalar.activation(
                    out_sb[:], hp[:], AF.Identity, bias=cst_sb[0:1, 0:1])
                nc.sync.dma_start(outp[:], out_sb[:])

            # --- final states --------------------------------------------
            nc.sync.dma_start(h1f[:], h1p3[:, :, 0:HID])
            nc.sync.dma_start(h2f[:], h2p[:])
            nc.sync.dma_start(c1f[:], c1p[:])
            nc.sync.dma_start(c2f[:], c2p[:])

    nc.compile()
    return nc


def _host_prep(x, W_ih0, W_hh0, b_ih0, b_hh0, W_ih1, W_hh1, b_ih1, b_hh1,
               w_lin, b_lin, w_end, b_end):
    """Pad to 128 nodes and build per-core input maps."""
    def pad_nodes(a):
        pad = [(0, 0)] * a.ndim
        pad[0] = (0, NP - N_NODES)
        return np.pad(a, pad)

    xp = np.pad(x, [(0, 0), (0, 0), (0, NP - N_NODES), (0, 0)])  # [B,T,NP,F]

    # reorder gate blocks [i, f, g, o] -> [i, f, o, g] so one device-side
    # sigmoid covers i,f,o contiguously
    gperm = np.r_[0:200, 300:400, 200:300]
    Wih0, Whh0 = pad_nodes(W_ih0), pad_nodes(W_hh0)
    Wih1, Whh1 = pad_nodes(W_ih1), pad_nodes(W_hh1)
    b0 = pad_nodes(b_ih0 + b_hh0)
    b1 = pad_nodes(b_ih1 + b_hh1)

    wlw = np.outer(w_lin[0], w_end[0]).astype(np.float32)  # [H, T]
    cst = np.array([[b_lin[0] * w_end[0].sum() + b_end[0]]], dtype=np.float32)
    ones = np.ones((1, PAIRS * 128), dtype=np.float32)

    in_maps = []
    for c in range(NCORES):
        sl = slice(c * NN, (c + 1) * NN)
        # xTp[t, f, p*128 + s*64 + b] = x[b, t, node, f], node = 16c+2p+s
        xTp = (xp[:, :, sl, :]                 # [B, T, 16, F]
               .transpose(1, 3, 2, 0)          # [T, F, 16, B]
               .reshape(SEQ, FEAT, PAIRS * 128))
        w0 = np.concatenate(
            [Whh0[sl].transpose(2, 0, 1), b0[sl][None],
             Wih0[sl].transpose(2, 0, 1)], axis=0)[:, :, gperm]
        w0 = w0.reshape(K0, NN * 400)
        w1 = np.concatenate(
            [Wih1[sl].transpose(2, 0, 1), b1[sl][None]], axis=0
        )[:, :, gperm].reshape(H1, NN * 400)
        wh2 = Whh1[sl].transpose(2, 0, 1)[:, :, gperm].reshape(HID, NN * 400)
        bf = ml_dtypes.bfloat16
        in_maps.append({
            "xTp": np.ascontiguousarray(xTp).astype(bf),
            "w0c": np.ascontiguousarray(w0).astype(bf),
            "w1c": np.ascontiguousarray(w1).astype(bf),
            "w1h2": np.ascontiguousarray(wh2).astype(bf),
            "wlw": wlw.astype(bf),
            "cst": cst,
            "ones_d": ones.astype(bf),
        })
    return in_maps


def kernel(x, W_ih0, W_hh0, b_ih0, b_hh0, W_ih1, W_hh1, b_ih1, b_hh1,
           w_lin, b_lin, w_end, b_end):
    global LAST_EXEC_TIME_NS
    args = (x, W_ih0, W_hh0, b_ih0, b_hh0, W_ih1, W_hh1, b_ih1, b_hh1,
            w_lin, b_lin, w_end, b_end)
    args = tuple(np.asarray(a, dtype=np.float32) for a in args)
    in_maps = _host_prep(*args)

    if "prog" not in _PROG_CACHE:
        _PROG_CACHE["prog"] = _build_program()
    nc = _PROG_CACHE["prog"]

    trace = os.environ.get("KERNEL_TRACE", "0") == "1"
    res = run_bass_kernel_spmd(
        nc, in_maps, core_ids=list(range(NCORES)), trace=trace
    )
    LAST_EXEC_TIME_NS = res.exec_time_ns

    out = np.zeros((B, 1, N_NODES, 1), dtype=np.float32)
    hn = np.zeros((N_NODES, 2, B, HID), dtype=np.float32)
    cn = np.zeros((N_NODES, 2, B, HID), dtype=np.float32)
    for c in range(NCORES):
        r = res.results[c]
        nlo, nhi = c * NN, min((c + 1) * NN, N_NODES)
        cnt = nhi - nlo

        # outp: [1, p*128 + s*64 + b] -> [node, b]
        o = r["outp"].reshape(PAIRS, 2, B).reshape(NN, B)
        out[:, 0, nlo:nhi, 0] = o[:cnt].T

        def states(a):  # [128, P*100] -> [node, b, h]
            v = a.reshape(2, B, PAIRS, HID)       # [s, b, p, h]
            return v.transpose(2, 0, 1, 3).reshape(NN, B, HID)

        hn[nlo:nhi, 0] = states(r["h1f"])[:cnt]
        hn[nlo:nhi, 1] = states(r["h2f"])[:cnt]
        cn[nlo:nhi, 0] = states(r["c1f"])[:cnt]
        cn[nlo:nhi, 1] = states(r["c2f"])[:cnt]
    return (out, hn, cn)


# revision 15
# speedup vs baseline: 5.9319x; 1.1474x over previous
"""Trainium2 Bass kernel for nn_AirspaceModel (126 per-node 2-layer LSTMs).

Sharding: 126 nodes padded to 128, 16 nodes per core across 8 cores
(expert-parallel over the independent per-node LSTMs). Each core keeps its
16 nodes' weights resident in SBUF and runs the full T=24 recurrence for
B=64.

Matmul orientation: gates[b, 4H] = xcatT.T @ W^T — the (transposed)
activations are the stationary operand [K<=128, 64] and the per-node bf16
weights stream as rhs [K, 400]. Nodes are processed in pairs: the even
node's matmul writes PSUM partitions 0-63 and the odd node's partitions
64-127 (tile_position=(0,64)), so the two matmuls run concurrently in
separate array column groups, elementwise ops see full 128-partition
tiles, and the tensor engine runs a pure dense matmul stream (which keeps
the HAM clock gate warm).

Recurrent states are bf16 [128, pair*128] tiles (batch-pair on
partitions; pair block = h(100) + ones col + pad). Each step the new
h1/h2 are transposed back to [*, 128] with ONE batched xbar DMA-transpose
per 4-pair group (zero tensor-engine cost); the layer-0 x rows are DMA'd
over the transpose's pad rows afterwards, and the ones column gives the
bias row that folds biases into the matmuls. Transposed h2 tiles for all
24 steps stay resident in SBUF (52KB) so the linear+conv head at the end
is a short chain of PSUM-accumulating matmuls against host-precomputed
w_lin[h]*w_end[t] columns with no DRAM round trip.

Gate columns are host-reordered to [i, f, o, g] so one sigmoid covers
i,f,o. PSUM accumulation and all elementwise math stay fp32 (cell states
fp32); only matmul operands and h are bf16.
"""

import os
import sys

import numpy as np
import ml_dtypes

for _p in ("/opt/trn_rl_repo", "/root/.axon_site/_ro/trn_rl_repo"):
    if os.path.isdir(_p) and _p not in sys.path:
        sys.path.append(_p)

import concourse.bass as bass
import concourse.mybir as mybir
import concourse.tile as tile
from concourse import bacc
from concourse.bass_utils import run_bass_kernel_spmd

F32 = mybir.dt.float32
BF16 = mybir.dt.bfloat16
AF = mybir.ActivationFunctionType

N_NODES, SEQ, FEAT, HID, B = 126, 24, 17, 100, 64
NCORES = 8
NP = 128             # padded node count
NN = NP // NCORES    # nodes per core (16)
PAIRS = NN // 2      # node pairs per core (8)
GP = 4               # pairs per pipeline group
NGRP = PAIRS // GP   # groups (2)
H1 = HID + 1         # h rows + ones col/row (101)
K0 = H1 + FEAT       # layer-0 contraction: h1 + ones + x = 118
PB = 128             # state pair-block width (h 0:100, ones 100, pad)

LAST_EXEC_TIME_NS = None
_PROG_CACHE = {}


def _build_program():
    nc = bacc.Bacc("TRN2", target_bir_lowering=False, debug=False)

    xTp = nc.dram_tensor("xTp", [SEQ, FEAT, PAIRS * 128], BF16, kind="ExternalInput")
    w0c = nc.dram_tensor("w0c", [K0, NN * 400], BF16, kind="ExternalInput")
    w1c = nc.dram_tensor("w1c", [H1, NN * 400], BF16, kind="ExternalInput")
    w1h2 = nc.dram_tensor("w1h2", [HID, NN * 400], BF16, kind="ExternalInput")
    wlw = nc.dram_tensor("wlw", [HID, SEQ], BF16, kind="ExternalInput")
    cst = nc.dram_tensor("cst", [1, 1], F32, kind="ExternalInput")
    ones_d = nc.dram_tensor("ones_d", [1, PAIRS * 128], BF16, kind="ExternalInput")

    outp = nc.dram_tensor("outp", [1, PAIRS * 128], F32, kind="ExternalOutput")
    h1f = nc.dram_tensor("h1f", [128, PAIRS * PB], BF16, kind="ExternalOutput")
    h2f = nc.dram_tensor("h2f", [128, PAIRS * PB], BF16, kind="ExternalOutput")
    c1f = nc.dram_tensor("c1f", [128, PAIRS * HID], F32, kind="ExternalOutput")
    c2f = nc.dram_tensor("c2f", [128, PAIRS * HID], F32, kind="ExternalOutput")

    with tile.TileContext(nc) as tc:
        with (
            tc.tile_pool(name="wpool", bufs=1) as wpool,
            tc.tile_pool(name="state", bufs=1) as state,
            tc.tile_pool(name="xc", bufs=3) as xc,
            tc.tile_pool(name="h2h", bufs=SEQ + 2) as h2h,
            tc.tile_pool(name="ew", bufs=2) as ew,
        ):
            # --- resident weights / constants ----------------------------
            w0c_sb = wpool.tile([K0, NN * 400], BF16)
            w1c_sb = wpool.tile([H1, NN * 400], BF16)
            w1h2_sb = wpool.tile([HID, NN * 400], BF16)
            wlw_sb = wpool.tile([HID, SEQ], BF16)
            cst_sb = wpool.tile([1, 1], F32)
            # chunked weight loads spread across the DMA dispatch engines
            # (sync/scalar HWDGE + gpsimd SWDGE) so they parallelize; the
            # scalar engine is idle during startup
            for lo in range(0, NN * 400, 4 * 400):
                hi = lo + 4 * 400
                nc.sync.dma_start(w0c_sb[:, lo:hi], w0c[:, lo:hi])
                nc.scalar.dma_start(w1c_sb[:, lo:hi], w1c[:, lo:hi])
                nc.gpsimd.dma_start(w1h2_sb[:, lo:hi], w1h2[:, lo:hi])
            nc.scalar.dma_start(wlw_sb[:], wlw[:])
            nc.scalar.dma_start(cst_sb[:], cst[:])

            # --- persistent state (batch-pair on partitions, bf16) -------
            # pair p occupies cols p*128..p*128+127: h 0:100, ones col 100
            h1p = state.tile([128, PAIRS * PB], BF16)
            h2p = state.tile([128, PAIRS * PB], BF16)
            c1p = state.tile([128, PAIRS * HID], F32)
            c2p = state.tile([128, PAIRS * HID], F32)
            nc.vector.memset(h1p[:], 0.0)
            nc.vector.memset(
                h1p.rearrange("p (n c) -> p n c", c=PB)[:, :, HID : H1], 1.0)
            nc.vector.memset(h2p[:], 0.0)
            nc.vector.memset(c1p[:], 0.0)
            nc.vector.memset(c2p[:], 0.0)

            # transposed stationary inputs, one [128, 8, 128] tile per step:
            # xh1 rows 0-99 h1T, 100 ones, 101-117 x_t; xh2 rows 0-99 h2T
            xh1_prev = xc.tile([128, PAIRS, 128], BF16, tag="xh1", name="xh1_init")
            nc.vector.memset(xh1_prev[:], 0.0)
            nc.sync.dma_start(
                xh1_prev[HID : H1].rearrange("o p b -> o (p b)"), ones_d[:])
            nc.sync.dma_start(
                xh1_prev[H1:K0].rearrange("o p b -> o (p b)"), xTp[0])

            def ewise(gl, hview, cview, sfx):
                """LSTM elementwise for a 4-pair group.

                gl: PSUM gates [128, 4, 512] (cols 0-399 used per pair,
                gate order i,f,o,g). hview: [128, 4, 100] bf16 h
                destination; cview: [128, 4, 100] fp32 cell-state home.
                """
                sf = ew.tile([128, GP * 300], F32, tag="sf" + sfx)
                sf3 = sf.rearrange("p (n c) -> p n c", n=GP)
                tg = ew.tile([128, GP * 100], F32, tag="tg" + sfx)
                tg3 = tg.rearrange("p (n c) -> p n c", n=GP)
                nc.scalar.activation(sf3[:], gl[:, :, 0:300], AF.Sigmoid)
                nc.scalar.activation(tg3[:], gl[:, :, 300:400], AF.Tanh)
                tmp1 = ew.tile([128, GP * 100], F32, tag="t1" + sfx)
                t13 = tmp1.rearrange("p (n c) -> p n c", n=GP)
                tmp2 = ew.tile([128, GP * 100], F32, tag="t2" + sfx)
                t23 = tmp2.rearrange("p (n c) -> p n c", n=GP)
                tcn = ew.tile([128, GP * 100], F32, tag="tc" + sfx)
                tc3 = tcn.rearrange("p (n c) -> p n c", n=GP)
                # c_new = sig(f)*c + sig(i)*tanh(g); the two products run
                # concurrently on different engines
                nc.vector.tensor_mul(t13[:], sf3[:, :, 100:200], cview)
                nc.gpsimd.tensor_mul(t23[:], sf3[:, :, 0:100], tg3[:])
                nc.vector.tensor_add(cview, t13[:], t23[:])
                nc.scalar.activation(tc3[:], cview, AF.Tanh)
                # h = sig(o)*tanh(c_new), cast to bf16 state
                nc.vector.tensor_mul(hview, sf3[:, :, 200:300], tc3[:])

            h1p3 = h1p.rearrange("p (n c) -> p n c", c=PB)
            h2p3 = h2p.rearrange("p (n c) -> p n c", c=PB)
            c1p3 = c1p.rearrange("p (n c) -> p n c", c=HID)
            c2p3 = c2p.rearrange("p (n c) -> p n c", c=HID)

            with tc.tile_pool(name="gates", bufs=2, space="PSUM") as gpsum:
                h2hist = []

                def transpose_h2(xh2_cur):
                    """Batched xbar-transpose of h2p into xh2_cur."""
                    for g in range(NGRP):
                        nc.sync.dma_start_transpose(
                            xh2_cur[0:128, g * GP : (g + 1) * GP, 0:128],
                            h2p[:, g * GP * PB : (g + 1) * GP * PB],
                        )

                for t in range(SEQ):
                    xh1_new = xc.tile([128, PAIRS, 128], BF16, tag="xh1",
                                      name=f"xh1_{t}")
                    xh2_cur = h2h.tile([128, PAIRS, 128], BF16, tag="xh2",
                                       name=f"xh2_{t}")
                    h2hist.append(xh2_cur)

                    # --- layer 0 matmuls (h1T(t-1) + ones + x(t)) --------
                    gl0 = []
                    for g in range(NGRP):
                        gl = gpsum.tile([128, GP, 512], F32, tag="g",
                                        name="gl0")
                        gl0.append(gl)
                        for j, p in enumerate(range(g * GP, (g + 1) * GP)):
                            for s in range(2):
                                n = 2 * p + s
                                nc.tensor.matmul(
                                    gl[s * 64 : (s + 1) * 64, j, 0:400],
                                    xh1_prev[0:K0, p, s * 64 : (s + 1) * 64],
                                    w0c_sb[:, n * 400 : (n + 1) * 400],
                                    start=True, stop=True,
                                    tile_position=(0, s * 64),
                                )
                    # --- h2T of the previous step ------------------------
                    if t == 0:
                        nc.vector.memset(xh2_cur[:], 0.0)
                    else:
                        transpose_h2(xh2_cur)
                    # --- layer 0 elementwise -----------------------------
                    for g in range(NGRP):
                        ewise(
                            gl0[g],
                            h1p3[:, g * GP : (g + 1) * GP, 0:HID],
                            c1p3[:, g * GP : (g + 1) * GP, :],
                            "a",
                        )
                    # --- transpose new h1 (+ones); then overlay x(t+1) ---
                    for g in range(NGRP):
                        nc.sync.dma_start_transpose(
                            xh1_new[0:128, g * GP : (g + 1) * GP, 0:128],
                            h1p[:, g * GP * PB : (g + 1) * GP * PB],
                        )
                    if t + 1 < SEQ:
                        nc.sync.dma_start(
                            xh1_new[H1:K0].rearrange("o p b -> o (p b)"),
                            xTp[t + 1])
                    # --- layer 1 matmuls ---------------------------------
                    gl1 = []
                    for g in range(NGRP):
                        gl = gpsum.tile([128, GP, 512], F32, tag="g",
                                        name="gl1")
                        gl1.append(gl)
                        for j, p in enumerate(range(g * GP, (g + 1) * GP)):
                            for s in range(2):
                                n = 2 * p + s
                                sl = slice(s * 64, (s + 1) * 64)
                                nc.tensor.matmul(
                                    gl[sl, j, 0:400],
                                    xh1_new[0:H1, p, sl],
                                    w1c_sb[:, n * 400 : (n + 1) * 400],
                                    start=True, stop=False,
                                    tile_position=(0, s * 64),
                                )
                                nc.tensor.matmul(
                                    gl[sl, j, 0:400],
                                    xh2_cur[0:HID, p, sl],
                                    w1h2_sb[:, n * 400 : (n + 1) * 400],
                                    start=False, stop=True,
                                    tile_position=(0, s * 64),
                                )
                    # --- layer 1 elementwise -----------------------------
                    for g in range(NGRP):
                        ewise(
                            gl1[g],
                            h2p3[:, g * GP : (g + 1) * GP, 0:HID],
                            c2p3[:, g * GP : (g + 1) * GP, :],
                            "b",
                        )
                    xh1_prev = xh1_new

                # final h2 transpose for the head (t = SEQ-1)
                xh2_last = h2h.tile([128, PAIRS, 128], BF16, tag="xh2",
                                    name="xh2_last")
                transpose_h2(xh2_last)
                h2hist.append(xh2_last)

            # --- head: out = sum_t wlw[:,t] . h2T(t) + cst ---------------
            # h2hist[t+1] holds h2T(t) (h2hist[0] is the zero init tile)
            with tc.tile_pool(name="hps", bufs=1, space="PSUM") as hps:
                hp = hps.tile([1, PAIRS * 128], F32)
                for t in range(SEQ):
                    h2c = h2hist[t + 1][0:HID].rearrange("o p b -> o (p b)")
                    for half in range(2):
                        nc.tensor.matmul(
                            hp[:, half * 512 : (half + 1) * 512],
                            wlw_sb[:, t : t + 1],
                            h2c[:, half * 512 : (half + 1) * 512],
                            start=(t == 0),
                            stop=(t == SEQ - 1),
                        )
                out_sb = ew.tile([1, PAIRS * 128], F32, tag="out_sb")
                nc.scalar.activation(
                    out_sb[:], hp[:], AF.Identity, bias=cst_sb[0:1, 0:1])
                nc.sync.dma_start(outp[:], out_sb[:])

            # --- final states --------------------------------------------
            nc.sync.dma_start(h1f[:], h1p[:])
            nc.sync.dma_start(h2f[:], h2p[:])
            nc.sync.dma_start(c1f[:], c1p[:])
            nc.sync.dma_start(c2f[:], c2p[:])

    nc.compile()
    return nc


def _host_prep(x, W_ih0, W_hh0, b_ih0, b_hh0, W_ih1, W_hh1, b_ih1, b_hh1,
               w_lin, b_lin, w_end, b_end):
    """Pad to 128 nodes and build per-core input maps."""
    def pad_nodes(a):
        pad = [(0, 0)] * a.ndim
        pad[0] = (0, NP - N_NODES)
        return np.pad(a, pad)

    xp = np.pad(x, [(0, 0), (0, 0), (0, NP - N_NODES), (0, 0)])  # [B,T,NP,F]

    # reorder gate blocks [i, f, g, o] -> [i, f, o, g] so one device-side
    # sigmoid covers i,f,o contiguously
    gperm = np.r_[0:200, 300:400, 200:300]
    Wih0, Whh0 = pad_nodes(W_ih0), pad_nodes(W_hh0)
    Wih1, Whh1 = pad_nodes(W_ih1), pad_nodes(W_hh1)
    b0 = pad_nodes(b_ih0 + b_hh0)
    b1 = pad_nodes(b_ih1 + b_hh1)

    wlw = np.outer(w_lin[0], w_end[0]).astype(np.float32)  # [H, T]
    cst = np.array([[b_lin[0] * w_end[0].sum() + b_end[0]]], dtype=np.float32)
    ones = np.ones((1, PAIRS * 128), dtype=np.float32)

    in_maps = []
    for c in range(NCORES):
        sl = slice(c * NN, (c + 1) * NN)
        # xTp[t, f, p*128 + s*64 + b] = x[b, t, node, f], node = 16c+2p+s
        xTp = (xp[:, :, sl, :]                 # [B, T, 16, F]
               .transpose(1, 3, 2, 0)          # [T, F, 16, B]
               .reshape(SEQ, FEAT, PAIRS * 128))
        w0 = np.concatenate(
            [Whh0[sl].transpose(2, 0, 1), b0[sl][None],
             Wih0[sl].transpose(2, 0, 1)], axis=0)[:, :, gperm]
        w0 = w0.reshape(K0, NN * 400)
        w1 = np.concatenate(
            [Wih1[sl].transpose(2, 0, 1), b1[sl][None]], axis=0
        )[:, :, gperm].reshape(H1, NN * 400)
        wh2 = Whh1[sl].transpose(2, 0, 1)[:, :, gperm].reshape(HID, NN * 400)
        bf = ml_dtypes.bfloat16
        in_maps.append({
            "xTp": np.ascontiguousarray(xTp).astype(bf),
            "w0c": np.ascontiguousarray(w0).astype(bf),
            "w1c": np.ascontiguousarray(w1).astype(bf),
            "w1h2": np.ascontiguousarray(wh2).astype(bf),
            "wlw": wlw.astype(bf),
            "cst": cst,
            "ones_d": ones.astype(bf),
        })
    return in_maps


def kernel(x, W_ih0, W_hh0, b_ih0, b_hh0, W_ih1, W_hh1, b_ih1, b_hh1,
           w_lin, b_lin, w_end, b_end):
    global LAST_EXEC_TIME_NS
    args = (x, W_ih0, W_hh0, b_ih0, b_hh0, W_ih1, W_hh1, b_ih1, b_hh1,
            w_lin, b_lin, w_end, b_end)
    args = tuple(np.asarray(a, dtype=np.float32) for a in args)
    in_maps = _host_prep(*args)

    if "prog" not in _PROG_CACHE:
        _PROG_CACHE["prog"] = _build_program()
    nc = _PROG_CACHE["prog"]

    trace = os.environ.get("KERNEL_TRACE", "0") == "1"
    res = run_bass_kernel_spmd(
        nc, in_maps, core_ids=list(range(NCORES)), trace=trace
    )
    LAST_EXEC_TIME_NS = res.exec_time_ns

    out = np.zeros((B, 1, N_NODES, 1), dtype=np.float32)
    hn = np.zeros((N_NODES, 2, B, HID), dtype=np.float32)
    cn = np.zeros((N_NODES, 2, B, HID), dtype=np.float32)
    for c in range(NCORES):
        r = res.results[c]
        nlo, nhi = c * NN, min((c + 1) * NN, N_NODES)
        cnt = nhi - nlo

        # outp: [1, p*128 + s*64 + b] -> [node, b]
        o = r["outp"].reshape(PAIRS, 2, B).reshape(NN, B)
        out[:, 0, nlo:nhi, 0] = o[:cnt].T

        def hstates(a):  # bf16 [128, P*128] -> [node, b, h]
            v = np.asarray(a, dtype=np.float32).reshape(2, B, PAIRS, PB)
            return v[..., 0:HID].transpose(2, 0, 1, 3).reshape(NN, B, HID)

        def cstates(a):  # f32 [128, P*100] -> [node, b, h]
            v = a.reshape(2, B, PAIRS, HID)
            return v.transpose(2, 0, 1, 3).reshape(NN, B, HID)

        hn[nlo:nhi, 0] = hstates(r["h1f"])[:cnt]
        hn[nlo:nhi, 1] = hstates(r["h2f"])[:cnt]
        cn[nlo:nhi, 0] = cstates(r["c1f"])[:cnt]
        cn[nlo:nhi, 1] = cstates(r["c2f"])[:cnt]
    return (out, hn, cn)
